# revision 1
# baseline (speedup 1.0000x reference)
"""Trainium2 Bass kernel for a 3-layer bidirectional GRU classifier.

Sharding: 8 cores = 4 batch shards (16 samples) x 2 directions.  Each core
runs only its own direction's recurrence (half the W_hh weight stream per
step) and computes its own direction's input projections (gi) for the next
layer.  Direction pairs exchange transposed hidden outputs per 64-step
window via pairwise AllGather so the gi GEMM can contract over the full
2H=2048 input (both directions).  Both cores of a pair compute the same
FC output for their shard; kernel() reads cores 0,2,4,6.
"""

import os
import sys

for _p in ("/opt/trn_rl_repo", "/root/.axon_site/_ro/trn_rl_repo"):
    if os.path.isdir(_p) and _p not in sys.path:
        sys.path.append(_p)

import numpy as np
import ml_dtypes

import concourse.bacc as bacc
import concourse.mybir as mybir
from concourse.tile import TileContext, add_dep_helper
from concourse.bass_utils import run_bass_kernel_spmd

BF16 = mybir.dt.bfloat16
F32 = mybir.dt.float32
AF = mybir.ActivationFunctionType
ALU = mybir.AluOpType

NCORES = 8
NSHARD = 4
H = 1024
G3 = 3 * H
CIN = 150
NCLS = 60
EPS = 1e-5
PAIRS = [[0, 1], [2, 3], [4, 5], [6, 7]]


def _perm3h(v):
    """Reorder a [..., 3072] gate-major vector to (a, gate, cj) column order."""
    a = v.reshape(*v.shape[:-1], 3, 4, 256)
    a = np.moveaxis(a, -3, -2)  # [..., 4, 3, 256]
    return a.reshape(*v.shape[:-1], G3)


def host_prep(inputs, T, n_full):
    B = n_full // NSHARD  # 16 samples per shard
    x = np.asarray(inputs["x"], np.float32)
    gamma = np.asarray(inputs["bn_gamma"], np.float32)
    beta = np.asarray(inputs["bn_beta"], np.float32)
    w_ih0 = np.asarray(inputs["w_ih0"], np.float32)
    w_hh0 = np.asarray(inputs["w_hh0"], np.float32)
    b_ih0 = np.asarray(inputs["b_ih0"], np.float32)
    b_hh0 = np.asarray(inputs["b_hh0"], np.float32)
    w_ih = np.asarray(inputs["w_ih"], np.float32)
    w_hh = np.asarray(inputs["w_hh"], np.float32)
    b_ih = np.asarray(inputs["b_ih"], np.float32)
    b_hh = np.asarray(inputs["b_hh"], np.float32)
    fc_w = np.asarray(inputs["fc_w"], np.float32)
    fc_b = np.asarray(inputs["fc_b"], np.float32)

    shared = {}
    NTF = T * n_full

    xTf = np.ascontiguousarray(x.transpose(2, 1, 0).reshape(CIN, NTF))
    shared["xtf0"] = np.ascontiguousarray(xTf[:128])
    xtf1 = np.zeros((32, NTF), np.float32)
    xtf1[: CIN - 128] = xTf[128:]
    shared["xtf1"] = xtf1

    # W_hh^T permuted bf16 per (layer, dir): [128, 8*3072], free=k*3072+a*768+g*256+cj
    whh_all = np.stack([w_hh0, w_hh[0], w_hh[1]])  # [3,2,3072,1024]
    t = whh_all.reshape(3, 2, 3, 4, 256, 8, 128)
    t = np.transpose(t, (0, 1, 6, 5, 3, 2, 4))
    whh_p = t.reshape(3, 2, 128, 8 * 3072).astype(ml_dtypes.bfloat16)

    # W_ih layers 1,2 per dir: [2, 16, 128, 3072]; k 0-7 = dir0 rows, 8-15 = dir1
    t = w_ih.reshape(2, 2, 3, 4, 256, 16, 128)
    t = np.transpose(t, (0, 5, 6, 1, 3, 2, 4))
    wih12_p = t.reshape(2, 16, 128, 2, 3072).astype(ml_dtypes.bfloat16)

    # W_ih0^T permuted fp32 [150->(128,32), 2, 3072]
    t = w_ih0.reshape(2, 3, 4, 256, CIN)
    t = np.transpose(t, (4, 0, 2, 1, 3)).reshape(CIN, 2, 3072)

    mask_rz = np.zeros(G3, np.float32)
    mask_rz[: 2 * H] = 1.0
    bias0_d = [_perm3h(b_ih0[d] + b_hh0[d] * mask_rz) for d in range(2)]
    b12_d = [[_perm3h(b_ih[li, d] + b_hh[li, d] * mask_rz) for li in range(2)]
             for d in range(2)]

    bhh_all = np.stack([b_hh0, b_hh[0], b_hh[1]])[:, :, 2 * H:]  # [3,2,1024]

    shared["fcwT"] = np.ascontiguousarray(
        fc_w.T.reshape(16, 128, NCLS).astype(ml_dtypes.bfloat16))
    shared["fcb"] = fc_b.reshape(1, NCLS).astype(ml_dtypes.bfloat16)

    shared["gamma"] = np.concatenate(
        [gamma, np.zeros(10, np.float32)]).reshape(160, 1)
    shared["beta"] = np.concatenate(
        [beta, np.zeros(10, np.float32)]).reshape(160, 1)

    shared["ones_bf"] = np.ones((1, 128), ml_dtypes.bfloat16)
    shared["ident128"] = np.eye(128, dtype=ml_dtypes.bfloat16)

    per_core = []
    for c in range(NCORES):
        s, d = c // 2, c % 2
        m = dict(shared)
        m["whh"] = np.ascontiguousarray(whh_p[:, d])  # [3, 128, 24576]
        m["wih12"] = np.ascontiguousarray(wih12_p[:, :, :, d])  # [2,16,128,3072]
        wd = np.ascontiguousarray(t[:, d])  # [150, 3072]
        m["wih0a"] = np.ascontiguousarray(wd[:128])
        w0b = np.zeros((32, 3072), np.float32)
        w0b[: CIN - 128] = wd[128:]
        m["wih0b"] = w0b
        m["bias0"] = bias0_d[d].reshape(1, 3072)
        m["bhhn"] = np.ascontiguousarray(
            bhh_all[:, d].reshape(3, 1, 1024).astype(ml_dtypes.bfloat16))
        m["bias12"] = np.stack(b12_d[d]).reshape(2, 1, 3072).astype(
            ml_dtypes.bfloat16)

        xo = x[s * B: (s + 1) * B]
        xT = xo.transpose(2, 1, 0).reshape(CIN, T * B)
        aug = np.zeros((160, T * B), np.float32)
        aug[:CIN] = xT
        aug[CIN] = 1.0
        m["xto"] = aug.astype(ml_dtypes.bfloat16)
        per_core.append(m)
    return per_core


def build_program(T, n_full):
    B = n_full // NSHARD  # 16
    ROWS = T * B
    NTF = T * n_full
    nc = bacc.Bacc("TRN2", target_bir_lowering=False, debug=False,
                   num_devices=NCORES)

    inp = {}
    def din(name, shape, dt):
        inp[name] = nc.dram_tensor(name, list(shape), dt, kind="ExternalInput")

    din("xtf0", (128, NTF), F32)
    din("xtf1", (32, NTF), F32)
    din("xto", (160, ROWS), BF16)
    din("whh", (3, 128, 8 * 3072), BF16)
    din("wih12", (2, 16, 128, 3072), BF16)
    din("wih0a", (128, 3072), F32)
    din("wih0b", (32, 3072), F32)
    din("bias0", (1, 3072), F32)
    din("bias12", (2, 1, 3072), BF16)
    din("bhhn", (3, 1, 1024), BF16)
    din("fcwT", (16, 128, NCLS), BF16)
    din("fcb", (1, NCLS), BF16)
    din("gamma", (160, 1), F32)
    din("beta", (160, 1), F32)
    din("ones_bf", (1, 128), BF16)
    din("ident128", (128, 128), BF16)

    out_t = nc.dram_tensor("out", [B, NCLS], F32, kind="ExternalOutput")

    WSTEPS = 50
    windows = []
    t0 = 0
    while t0 < T:
        windows.append((t0, min(t0 + WSTEPS, T)))
        t0 = min(t0 + WSTEPS, T)

    def mchunks(r0, r1):
        out = []
        while r0 < r1:
            out.append((r0, min(r0 + 128, r1)))
            r0 = min(r0 + 128, r1)
        return out

    with TileContext(nc) as tc:
        from contextlib import ExitStack
        ctx = ExitStack()
        pers = ctx.enter_context(tc.tile_pool(name="pers", bufs=1))
        gates_pool = ctx.enter_context(
            tc.tile_pool(name="gates", bufs=2, space="PSUM"))
        tp_pool = ctx.enter_context(
            tc.tile_pool(name="tpsum", bufs=1, space="PSUM"))
        gi_psum_pool = ctx.enter_context(
            tc.tile_pool(name="gipsum", bufs=3, space="PSUM"))
        dram_pool = ctx.enter_context(
            tc.tile_pool(name="dram", bufs=1, space="DRAM"))
        xch_pool = ctx.enter_context(
            tc.tile_pool(name="xch", bufs=2, space="DRAM"))
        gld_pool = ctx.enter_context(tc.tile_pool(name="gld", bufs=2))
        gicp_pool = ctx.enter_context(tc.tile_pool(name="gicp", bufs=2))

        ident128 = pers.tile([128, 128], BF16, tag="ident128")
        nc.sync.dma_start(ident128[:], inp["ident128"][:])
        ones_bf = pers.tile([1, 128], BF16, tag="ones")
        nc.sync.dma_start(ones_bf[:], inp["ones_bf"][:])

        gi_rz = [dram_pool.tile([T, 4, B, 512], BF16, tag=f"girz{l}",
                                name=f"gi_rz{l}") for l in range(3)]
        gi_n = [dram_pool.tile([T, 4, B, 256], BF16, tag=f"gin{l}",
                               name=f"gi_n{l}") for l in range(3)]

        def store_gi(l, g, r0, r1, cc0, cw, gsb):
            mw = r1 - r0
            if cc0 < 512:
                dstt = gi_rz[l][r0 // B: r1 // B, g][:, :, cc0: cc0 + cw]
            else:
                dstt = gi_n[l][r0 // B: r1 // B, g]
            nc.sync.dma_start(dstt, gsb[0:mw, 0:cw])

        # ---------------- phase 0: BN stats ----------------
        stats = []
        with tc.tile_pool(name="ph0s", bufs=1) as ph0s:
            for si, p in ((0, 128), (1, 32)):
                st = ph0s.tile([p, 32], F32, tag=f"st{si}", name=f"st{si}")
                stats.append(st)
                xt = ph0s.tile([p, NTF], F32, tag=f"xt{si}", name=f"xt{si}")
                nc.sync.dma_start(xt[:], inp[f"xtf{si}"][:])
                C = lambda i: st[:, i:i+1]
                nc.vector.tensor_reduce(C(0), xt[:],
                                        axis=mybir.AxisListType.X, op=ALU.add)
                nc.scalar.activation(xt[:], xt[:], AF.Square, accum_out=C(1))
                nc.vector.tensor_scalar_mul(C(2), C(0), 1.0 / NTF)
                nc.vector.tensor_scalar_mul(C(3), C(1), 1.0 / NTF)
                nc.vector.tensor_mul(C(4), C(2), C(2))
                nc.vector.tensor_sub(C(5), C(3), C(4))
                nc.vector.tensor_scalar_add(C(5), C(5), EPS)
                nc.scalar.activation(C(6), C(5), AF.Sqrt)
                nc.vector.reciprocal(C(7), C(6))
                nc.vector.tensor_mul(C(8), C(7), C(7))
                nc.vector.tensor_mul(C(9), C(5), C(8))
                nc.vector.scalar_tensor_tensor(
                    C(10), C(9), -0.5, C(7), op0=ALU.mult, op1=ALU.mult)
                nc.vector.scalar_tensor_tensor(
                    C(11), C(7), 1.5, C(10), op0=ALU.mult, op1=ALU.add)
                nc.sync.dma_start(C(12), inp["gamma"][si*128: si*128+p, :])
                nc.sync.dma_start(C(13), inp["beta"][si*128: si*128+p, :])
                nc.vector.tensor_mul(C(14), C(12), C(11))
                nc.vector.tensor_mul(C(15), C(2), C(14))
                nc.vector.tensor_sub(C(16), C(13), C(15))
            stp = [pers.tile([p_, 32], F32, tag=f"stp{si_}", name=f"stp{si_}")
                   for si_, p_ in ((0, 128), (1, 32))]
            for si in range(2):
                nc.vector.tensor_copy(stp[si][:, 0:17], stats[si][:, 0:17])

        # ---------------- phase 0b: W0 fold + gi0 (own dir) ---------
        with tc.tile_pool(name="ph0w", bufs=1) as ph0w:
            w0a = ph0w.tile([128, 3072], F32, tag="w0a", name="w0a")
            nc.sync.dma_start(w0a[:], inp["wih0a"][:])
            w0b = ph0w.tile([32, 3072], F32, tag="w0b", name="w0b")
            nc.sync.dma_start(w0b[:], inp["wih0b"][:])
            bias0 = ph0w.tile([1, 3072], F32, tag="bias0t", name="bias0t")
            nc.sync.dma_start(bias0[:], inp["bias0"][:])

            w0rows = [(w0a, 128, stp[0]), (w0b, CIN - 128, stp[1])]
            for n in range(6):
                bps = gi_psum_pool.tile([128, 512], F32, tag="gips",
                                        name=f"bps{n}")
                for ki, (w0, kp, st) in enumerate(w0rows):
                    nc.tensor.matmul(
                        bps[0:1, 0:512], st[0:kp, 16:17],
                        w0[0:kp, n * 512: (n + 1) * 512],
                        start=(ki == 0), stop=(ki == 1))
                nc.vector.tensor_add(bias0[:, n * 512: (n + 1) * 512],
                                     bps[0:1, 0:512],
                                     bias0[:, n * 512: (n + 1) * 512])
            for w0, kp, st in w0rows:
                nc.vector.tensor_scalar_mul(w0[0:kp, :], w0[0:kp, :],
                                            st[0:kp, 14:15])
            nc.sync.dma_start(w0b[CIN - 128: CIN - 128 + 1, :], bias0[:])
            w0ab = ph0w.tile([128, 3072], BF16, tag="w0ab", name="w0ab")
            w0bb = ph0w.tile([32, 3072], BF16, tag="w0bb", name="w0bb")
            nc.vector.tensor_copy(w0ab[:], w0a[:])
            nc.vector.tensor_copy(w0bb[0: CIN + 1 - 128, :],
                                  w0b[0: CIN + 1 - 128, :])

            xto_a = ph0w.tile([128, ROWS], BF16, tag="xtoa", name="xtoa")
            nc.sync.dma_start(xto_a[:], inp["xto"][0:128, :])
            xto_b = ph0w.tile([32, ROWS], BF16, tag="xtob", name="xtob")
            nc.sync.dma_start(xto_b[:], inp["xto"][128:160, :])

            for (r0, r1) in mchunks(0, ROWS):
                mw = r1 - r0
                for g in range(4):
                    for (kind, cc0, cw) in (("rz", 0, 512), ("n", 512, 256)):
                        gps = gi_psum_pool.tile(
                            [128, 512], F32, tag="gips",
                            name=f"g0ps{r0}_{g}_{kind}")
                        nc.tensor.matmul(
                            gps[0:mw, 0:cw],
                            xto_a[:, r0:r1],
                            w0ab[:, g * 768 + cc0: g * 768 + cc0 + cw],
                            start=True, stop=False)
                        nc.tensor.matmul(
                            gps[0:mw, 0:cw],
                            xto_b[0: CIN + 1 - 128, r0:r1],
                            w0bb[0: CIN + 1 - 128,
                                 g * 768 + cc0: g * 768 + cc0 + cw],
                            start=False, stop=True)
                        gsb = gicp_pool.tile(
                            [128, cw], BF16, tag=f"gisb_{kind}",
                            name=f"g0sb{r0}_{g}_{kind}")
                        nc.vector.tensor_copy(gsb[0:mw, :], gps[0:mw, 0:cw])
                        store_gi(0, g, r0, r1, cc0, cw, gsb)

        # ---------------- layers ----------------
        scan_pool = ctx.enter_context(tc.tile_pool(name="scan", bufs=1))
        owin_pool = ctx.enter_context(tc.tile_pool(name="owin", bufs=2))
        og_pool = ctx.enter_context(tc.tile_pool(name="og", bufs=2))
        outT_last = scan_pool.tile([128, 2 * 8 * B], BF16, tag="outTlast",
                                   name="outT_last")

        def exchange_window(l, ts, te, owin):
            """AllGather own outT window with pair core.

            Returns (og0, og1) gathered window tiles for l<2; fills
            outT_last for l==2 (last window only)."""
            wsz = (te - ts) * B
            tagsfx = f"{l}_{ts}"
            xin = xch_pool.tile([128, 8 * wsz], BF16, tag="xin",
                                name=f"xin_{tagsfx}")
            nc.sync.dma_start(xin[:], owin[:, 0: 8 * wsz])
            xout = xch_pool.tile([2, 128, 8 * wsz], BF16, tag="xout",
                                 name=f"xout_{tagsfx}")
            nc.gpsimd.collective_compute(
                "AllGather", ALU.bypass, replica_groups=PAIRS,
                ins=[xin[:].opt()], outs=[xout[:].opt()])
            if l == 2:
                for d in range(2):
                    nc.sync.dma_start(
                        outT_last.rearrange("p (d k r) -> p d k r",
                                            d=2, k=8)[:, d],
                        xout[d].rearrange("p (k r) -> p k r", k=8)[
                            :, :, wsz - B: wsz])
                return None
            og = [og_pool.tile([128, 8 * wsz], BF16, tag=f"ogd{d}",
                               name=f"og{d}_{tagsfx}") for d in range(2)]
            for d in range(2):
                nc.sync.dma_start(og[d][:, 0: 8 * wsz], xout[d])
            return og

        wih_pool = ctx.enter_context(tc.tile_pool(name="wih", bufs=2))

        def gi_jobs(l, ts, te, og):
            """Jobs for the layer-(l+1) input-projection GEMM of a window.

            Returns a list of closures; each emits one PE-sized chunk of
            work.  Weight-load jobs are interleaved one group ahead of the
            matmul jobs that consume them (wih_pool bufs=2 double-buffer)."""
            wsz = (te - ts) * B
            groups = [(g, kind, cc0, cw)
                      for g in range(4)
                      for (kind, cc0, cw) in (("rz", 0, 512),
                                              ("n", 512, 256))]
            tiles = {}

            def load_job(gi_idx):
                g, kind, cc0, cw = groups[gi_idx]
                col = g * 768 + cc0
                wt = wih_pool.tile([128, 16 * cw], BF16, tag="wihT",
                                   name=f"wt_{l}{ts}{g}{kind}")
                for k in range(16):
                    nc.sync.dma_start(
                        wt[:, k * cw: (k + 1) * cw],
                        inp["wih12"][l, k, :, col: col + cw])
                bt = wih_pool.tile([1, cw], BF16, tag="biasT",
                                   name=f"bt_{l}{ts}{g}{kind}")
                nc.sync.dma_start(bt[:], inp["bias12"][l, :, col: col + cw])
                tiles[gi_idx] = (wt, bt)

            def mm_job(gi_idx, r0, r1):
                g, kind, cc0, cw = groups[gi_idx]
                wt, bt = tiles[gi_idx]
                mw = r1 - r0
                q0 = r0 - ts * B
                gps = gi_psum_pool.tile(
                    [128, 512], F32, tag="gips",
                    name=f"gp_{l}{ts}{g}{kind}{r0}")
                for k in range(16):
                    dsrc, kk = k // 8, k % 8
                    nc.tensor.matmul(
                        gps[0:mw, 0:cw],
                        og[dsrc][:, kk * wsz + q0: kk * wsz + q0 + mw],
                        wt[:, k * cw: (k + 1) * cw],
                        start=(k == 0), stop=False)
                nc.tensor.matmul(
                    gps[0:mw, 0:cw], ones_bf[:, 0:mw], bt[:],
                    start=False, stop=True)

                def copy_out():
                    gsb = gicp_pool.tile(
                        [128, cw], BF16, tag="gisb",
                        name=f"gs_{l}{ts}{g}{kind}{r0}")
                    nc.vector.tensor_copy(gsb[0:mw, :], gps[0:mw, 0:cw])
                    store_gi(l + 1, g, r0, r1, cc0, cw, gsb)
                copyout_q.append(copy_out)

            jobs = []
            rcs = mchunks(ts * B, te * B)
            jobs.append(lambda: load_job(0))
            for gi_idx in range(len(groups)):
                if gi_idx + 1 < len(groups):
                    jobs.append(lambda i=gi_idx + 1: load_job(i))
                for (r0, r1) in rcs:
                    jobs.append(lambda i=gi_idx, a=r0, b=r1: mm_job(i, a, b))
            return jobs

        SCH = 8  # steps per gi-load DMA chunk
        pending = []  # gi jobs awaiting a PE gap
        copyout_q = []  # deferred gi PSUM->SBUF copy-outs
        for l in range(3):
            whh_sb = scan_pool.tile([128, 8 * 3072], BF16, tag="whh_sb",
                                    name=f"whh_sb{l}")
            for q in range(4):
                nc.sync.dma_start(whh_sb[:, q * 6144: (q + 1) * 6144],
                                  inp["whh"][l][:, q * 6144: (q + 1) * 6144])
            bhhn_sb = scan_pool.tile([1, 1024], BF16, tag="bhhn_sb",
                                     name=f"bhhn_sb{l}")
            nc.sync.dma_start(bhhn_sb[:], inp["bhhn"][l])

            zlhs = scan_pool.tile([128, B], BF16, tag="zlhs", name=f"zlhs{l}")
            nc.vector.memset(zlhs[:], 0.0)
            h_elem = [scan_pool.tile([128, 256], BF16, tag=f"h_{par}",
                                     name=f"h_{par}_{l}") for par in range(2)]
            nc.vector.memset(h_elem[0][:], 0.0)

            # scratch (bf16): rz 0:512, tb 512:768, t1 768:1024, t2 1024:1280,
            # nt 1280:1536, dd 1536:1792, zd 1792:2048
            scr = scan_pool.tile([128, 2048], BF16, tag="scr", name=f"scr{l}")

            owin_prev, wsz_prev = None, 0
            for (ts, te) in windows:
                wsz = (te - ts) * B
                nsteps = te - ts
                owin = owin_pool.tile([128, 8 * wsz], BF16, tag="owin",
                                      name=f"owin_{l}_{ts}")
                emitted = 0
                npend0 = len(pending)

                def load_chunk(t0):
                    tS = min(SCH, te - t0)
                    grz = gld_pool.tile([128, SCH * 512], BF16, tag="grz",
                                        name=f"grz_{l}_{t0}")
                    gst = gld_pool.tile([128, SCH * 256], BF16, tag="gst",
                                        name=f"gst_{l}_{t0}")
                    for g in range(4):
                        nc.sync.dma_start(
                            grz[32*g: 32*g + B, 0: tS * 512],
                            gi_rz[l][t0: t0 + tS, g].rearrange(
                                "s b c -> b s c"))
                        nc.sync.dma_start(
                            gst[32*g: 32*g + B, 0: tS * 256],
                            gi_n[l][t0: t0 + tS, g].rearrange(
                                "s b c -> b s c"))
                    return grz, gst

                def preload(t0, grz, after=None):
                    # gi_rz into the psum accumulation region (ScalarE:
                    # keeps the DVE queue free for the activation chain)
                    gp = gates_pool.tile([128, 768], F32, tag="gp",
                                         name=f"gp_{l}_{t0}")
                    so = (t0 - ts) % SCH
                    cp = nc.scalar.copy(gp[:, 0:512],
                                        grz[:, so * 512: (so + 1) * 512])
                    if after is not None:
                        # keep the copy out of the ACT queue ahead of the
                        # current step's activation chain
                        add_dep_helper(cp.ins, after.ins, sync=False,
                                       reason="preload after chain")
                    return gp

                grz, gst = load_chunk(ts)
                gp_next = preload(ts, grz)
                for t in range(ts, te):
                    so = (t - ts) % SCH
                    gp = gp_next
                    for k in range(8):
                        if t == 0:
                            lhsT = zlhs[:, 0:B]
                        elif t == ts:
                            lhsT = owin_prev[:, k * wsz_prev
                                             + wsz_prev - B:
                                             k * wsz_prev + wsz_prev]
                        else:
                            lhsT = owin[:, k * wsz + (t - 1 - ts) * B:
                                        k * wsz + (t - ts) * B]
                        for (c0, cw) in ((0, 512), (512, 256)):
                            for g in range(4):
                                nc.tensor.matmul(
                                    gp[32*g: 32*g + B, c0: c0 + cw],
                                    lhsT,
                                    whh_sb[:, k * 3072 + g * 768 + c0:
                                           k * 3072 + g * 768 + c0 + cw],
                                    start=(c0 == 512 and k == 0),
                                    stop=(c0 == 0 and k == 7),
                                    skip_group_check=True,
                                    tile_position=(0, 32 * g))
                    for g in range(4):
                        nc.tensor.matmul(
                            gp[32*g: 32*g + B, 512:768],
                            ones_bf[:, 0:B],
                            bhhn_sb[:, g * 256: (g+1) * 256],
                            start=False, stop=True,
                            skip_group_check=True,
                            tile_position=(0, 32 * g))

                    # drain pending gi jobs into the PE gap after this
                    # step's matmuls (skip early steps so the AllGather
                    # of the previous window has landed); only PE work +
                    # weight DMAs are emitted here — their DVE copy-outs
                    # flush at the end of the step
                    SKIP = 4
                    nemit = 0
                    if t - ts >= SKIP:
                        frac = (t - ts - SKIP + 1) / max(nsteps - SKIP, 1)
                        target = min(npend0, int(frac * npend0 + 0.999))
                        while emitted < target and pending:
                            pending.pop(0)()
                            emitted += 1
                            nemit += 1
                    if nemit == 0 and t - ts >= 2:
                        # queue is dry this step: issue a throwaway matmul
                        # so the PE activity monitor doesn't throttle the
                        # clock during the activation-chain gap
                        wps = gi_psum_pool.tile([128, 512], F32, tag="gips",
                                                name=f"warm_{l}_{t}")
                        for wq in range(3):
                            nc.tensor.matmul(
                                wps[0:B, 0:512], zlhs[:, 0:B],
                                whh_sb[:, wq * 512: (wq + 1) * 512],
                                start=(wq == 0), stop=(wq == 2))

                    h_prev = h_elem[t % 2]
                    h_new = h_elem[(t + 1) % 2]
                    rz = scr[:, 0:512]
                    omz = scr[:, 512:768]
                    t1, t2 = scr[:, 768:1024], scr[:, 1024:1280]
                    nt, zh = scr[:, 1280:1536], scr[:, 1536:1792]
                    u = scr[:, 1792:2048]
                    nc.scalar.activation(rz, gp[:, 0:512], AF.Sigmoid)
                    nc.vector.tensor_mul(t1, rz[:, 0:256], gp[:, 512:768])
                    nc.vector.tensor_add(t2, t1,
                                         gst[:, so * 256: (so + 1) * 256])
                    tanh_inst = nc.scalar.activation(nt, t2, AF.Tanh)
                    # (1-z) and z*h_prev computed while the tanh runs, so
                    # only two DVE ops remain after it
                    nc.vector.tensor_scalar(omz, rz[:, 256:512], -1.0, 1.0,
                                            op0=ALU.mult, op1=ALU.add)
                    nc.vector.tensor_mul(zh, rz[:, 256:512], h_prev[:])
                    nc.vector.tensor_mul(u, omz, nt)
                    nc.vector.tensor_add(h_new[:], u, zh)

                    dst = owin.rearrange(
                        "p (g j2 r) -> p g j2 r", g=4, j2=2)[
                        :, :, :, (t - ts) * B: (t - ts + 1) * B]
                    tp = tp_pool.tile([128, 256], BF16, tag="tp",
                                      name=f"tp_{l}_{t}")
                    src = tp.rearrange(
                        "p (j2 g b) -> p g j2 b", j2=2, g=4)[:, :, :, 0:B]
                    for j2 in range(2):
                        nc.tensor.transpose(
                            tp[:, j2 * 128: (j2 + 1) * 128],
                            h_new[:, j2 * 128: (j2 + 1) * 128],
                            ident128[:])
                        nc.vector.tensor_copy(dst[:, :, j2], src[:, :, j2])

                    # pipeline the next step's gi-load chunk + psum preload
                    if t + 1 < te:
                        if (t + 1 - ts) % SCH == 0:
                            grz, gst = load_chunk(t + 1)
                        gp_next = preload(t + 1, grz, after=tanh_inst)
                    while copyout_q:
                        copyout_q.pop(0)()

                # end of window: ship it; its gi jobs drain during later
                # windows' step gaps
                if l < 2 or te == T:
                    og = exchange_window(l, ts, te, owin)
                if l < 2:
                    pending.extend(gi_jobs(l, ts, te, og))
                owin_prev, wsz_prev = owin, wsz

        # flush any gi jobs not drained during step gaps
        while pending:
            pending.pop(0)()
        while copyout_q:
            copyout_q.pop(0)()

        # ---------------- FC ----------------
        fcw = pers.tile([128, 16 * NCLS], BF16, tag="fcw")
        for k in range(16):
            nc.sync.dma_start(fcw[:, k * NCLS: (k + 1) * NCLS],
                              inp["fcwT"][k])
        fcb = pers.tile([1, NCLS], BF16, tag="fcb")
        nc.sync.dma_start(fcb[:], inp["fcb"][:])
        fps = gi_psum_pool.tile([128, 512], F32, tag="gips", name="fps")
        for k in range(16):
            d, kk = k // 8, k % 8
            nc.tensor.matmul(
                fps[0:B, 0:NCLS],
                outT_last[:, (d * 8 + kk) * B: (d * 8 + kk + 1) * B],
                fcw[:, k * NCLS: (k + 1) * NCLS],
                start=(k == 0), stop=False)
        nc.tensor.matmul(fps[0:B, 0:NCLS], ones_bf[:, 0:B], fcb[:],
                         start=False, stop=True)
        fout = gicp_pool.tile([B, NCLS], F32, tag="fout")
        nc.vector.tensor_copy(fout[:], fps[0:B, 0:NCLS])
        nc.sync.dma_start(out_t[:], fout[:])

        ctx.close()

    nc.compile()
    return nc


_cache = {}


def kernel(**inputs):
    T = inputs["x"].shape[1]
    n_full = inputs["x"].shape[0]
    key = ("prog", T, n_full)
    if key not in _cache:
        _cache[key] = build_program(T, n_full)
    nc = _cache[key]
    per_core = host_prep(inputs, T, n_full)
    res = run_bass_kernel_spmd(nc, per_core, core_ids=list(range(NCORES)))
    out = np.concatenate([res.results[2 * s]["out"] for s in range(NSHARD)],
                         axis=0)
    return np.ascontiguousarray(out.astype(np.float32))



# revision 21
# speedup vs baseline: 1.0942x; 1.0942x over previous
"""Trainium2 Bass kernel for a 3-layer bidirectional GRU classifier.

Sharding: 8 cores = 4 batch shards (16 samples) x 2 directions.  Each core
runs only its own direction's recurrence and computes its own direction's
input projections (gi) for the next layer.  Direction pairs exchange
transposed hidden outputs per window via pairwise AllGather so the gi GEMM
can contract over the full 2H=2048 input.  Both cores of a pair compute the
same FC output for their shard; kernel() reads cores 0,2,4,6.

h^T layout (owin): produced per step by one DVE StreamTranspose of
h_new [128=4a x 32(b<16), 256=cj] -> owin[32a+m, j*32+b] = h[b, a*256+j*32+m]
so rec lhsT k-chunk j is owin[:, s*256 + j*32 : +16].  Weights are permuted
host-side to match.
"""

import os
import sys

for _p in ("/opt/trn_rl_repo", "/root/.axon_site/_ro/trn_rl_repo"):
    if os.path.isdir(_p) and _p not in sys.path:
        sys.path.append(_p)

import numpy as np
import ml_dtypes

import concourse.bacc as bacc
import concourse.mybir as mybir
from concourse.tile import TileContext
from concourse.bass_utils import run_bass_kernel_spmd

BF16 = mybir.dt.bfloat16
F32 = mybir.dt.float32
AF = mybir.ActivationFunctionType
ALU = mybir.AluOpType

NCORES = 8
NSHARD = 4
H = 1024
G3 = 3 * H
CIN = 150
NCLS = 60
EPS = 1e-5
PAIRS = [[0, 1], [2, 3], [4, 5], [6, 7]]

WSTEPS = 30
SCH = 8
GAP_NS = 2600.0
DEBUG_EMIT = False
SIM_SAFE = False  # set True for MultiCoreSim runs (init junk lanes)


def MMC(cw):
    return cw / 2.4 + 50.0


def _perm3h(v):
    """Reorder a [..., 3072] gate-major vector to (a, gate, cj) column order."""
    a = v.reshape(*v.shape[:-1], 3, 4, 256)
    a = np.moveaxis(a, -3, -2)  # [..., 4, 3, 256]
    return a.reshape(*v.shape[:-1], G3)


def host_prep(inputs, T, n_full):
    B = n_full // NSHARD  # 16 samples per shard
    x = np.asarray(inputs["x"], np.float32)
    gamma = np.asarray(inputs["bn_gamma"], np.float32)
    beta = np.asarray(inputs["bn_beta"], np.float32)
    w_ih0 = np.asarray(inputs["w_ih0"], np.float32)
    w_hh0 = np.asarray(inputs["w_hh0"], np.float32)
    b_ih0 = np.asarray(inputs["b_ih0"], np.float32)
    b_hh0 = np.asarray(inputs["b_hh0"], np.float32)
    w_ih = np.asarray(inputs["w_ih"], np.float32)
    w_hh = np.asarray(inputs["w_hh"], np.float32)
    b_ih = np.asarray(inputs["b_ih"], np.float32)
    b_hh = np.asarray(inputs["b_hh"], np.float32)
    fc_w = np.asarray(inputs["fc_w"], np.float32)
    fc_b = np.asarray(inputs["fc_b"], np.float32)

    shared = {}
    NTF = T * n_full

    xTf = np.ascontiguousarray(x.transpose(2, 1, 0).reshape(CIN, NTF))
    shared["xtf0"] = np.ascontiguousarray(xTf[:128])
    xtf1 = np.zeros((32, NTF), np.float32)
    xtf1[: CIN - 128] = xTf[128:]
    shared["xtf1"] = xtf1

    # W_hh^T per (layer, dir): [128=ai*32+m, 8j*3072] with in-dim
    # d = ai*256 + j*32 + m, cols (j, a', gate, cj)
    whh_all = np.stack([w_hh0, w_hh[0], w_hh[1]])  # [3,2,3072,1024]
    t = whh_all.reshape(3, 2, 3, 4, 256, 4, 8, 32)  # l d g3 a' cj ai j m
    t = np.transpose(t, (0, 1, 5, 7, 6, 3, 2, 4))  # l d ai m j a' g3 cj
    whh_p = np.ascontiguousarray(t.reshape(3, 2, 128, 8 * 3072)).astype(
        ml_dtypes.bfloat16)

    # W_ih layers 1,2: [l, k=(dsrc*8+j), q=ai*32+m, dir, col(a',gate,cj)]
    t = w_ih.reshape(2, 2, 3, 4, 256, 2, 4, 8, 32)  # l d g3 a' cj ds ai j m
    t = np.transpose(t, (0, 5, 7, 6, 8, 1, 3, 2, 4))  # l ds j ai m d a' g3 cj
    wih12_p = t.reshape(2, 16, 128, 2, 3072).astype(ml_dtypes.bfloat16)

    # W_ih0^T [150->(128,32), 2, 3072] (contraction over CIN: unaffected)
    t0 = w_ih0.reshape(2, 3, 4, 256, CIN)
    t0 = np.transpose(t0, (4, 0, 2, 1, 3)).reshape(CIN, 2, 3072)

    mask_rz = np.zeros(G3, np.float32)
    mask_rz[: 2 * H] = 1.0
    bias0_d = [_perm3h(b_ih0[d] + b_hh0[d] * mask_rz) for d in range(2)]
    b12_d = [[_perm3h(b_ih[li, d] + b_hh[li, d] * mask_rz) for li in range(2)]
             for d in range(2)]

    bhh_all = np.stack([b_hh0, b_hh[0], b_hh[1]])[:, :, 2 * H:]  # [3,2,1024]

    # FC: fcwT[k=(d*8+j)][q=ai*32+m, c]
    fw = fc_w.reshape(NCLS, 2, 4, 8, 32)  # c d ai j m
    fw = np.transpose(fw, (1, 3, 2, 4, 0))  # d j ai m c
    shared["fcwT"] = np.ascontiguousarray(
        fw.reshape(16, 128, NCLS).astype(ml_dtypes.bfloat16))
    shared["fcb"] = fc_b.reshape(1, NCLS).astype(ml_dtypes.bfloat16)

    shared["gamma"] = np.concatenate(
        [gamma, np.zeros(10, np.float32)]).reshape(160, 1)
    shared["beta"] = np.concatenate(
        [beta, np.zeros(10, np.float32)]).reshape(160, 1)

    shared["ones_bf"] = np.ones((1, 128), ml_dtypes.bfloat16)
    idr = np.zeros((128, 32), np.float32)
    for g in range(4):
        for i in range(16):
            idr[32 * g + i, i] = 1.0
            idr[32 * g + i, 16 + i] = 1.0
    shared["identrep"] = idr.astype(ml_dtypes.bfloat16)

    per_core = []
    for c in range(NCORES):
        s, d = c // 2, c % 2
        m = dict(shared)
        m["whh"] = np.ascontiguousarray(whh_p[:, d])  # [3, 128, 24576]
        m["wih12"] = np.ascontiguousarray(wih12_p[:, :, :, d])  # [2,16,128,3072]
        wd = np.ascontiguousarray(t0[:, d])  # [150, 3072]
        m["wih0a"] = np.ascontiguousarray(wd[:128])
        w0b = np.zeros((32, 3072), np.float32)
        w0b[: CIN - 128] = wd[128:]
        m["wih0b"] = w0b
        m["bias0"] = bias0_d[d].reshape(1, 3072)
        m["bhhn"] = np.ascontiguousarray(
            bhh_all[:, d].reshape(3, 1, 1024).astype(ml_dtypes.bfloat16))
        m["bias12"] = np.stack(b12_d[d]).reshape(2, 1, 3072).astype(
            ml_dtypes.bfloat16)

        xo = x[s * B: (s + 1) * B]
        xT = xo.transpose(2, 1, 0).reshape(CIN, T * B)
        aug = np.zeros((160, T * B), np.float32)
        aug[:CIN] = xT
        aug[CIN] = 1.0
        m["xto"] = aug.astype(ml_dtypes.bfloat16)
        per_core.append(m)
    return per_core


def _waterfill(batches, caps):
    """batches: list of (cost, avail, dl). Returns per-window alloc list."""
    n = len(caps)
    alloc = [0.0] * n
    rem = list(caps)
    for cost, avail, dl in batches:
        lo = max(0, min(avail, n - 1))
        hi = max(lo, min(dl - 1, n - 1))
        left = float(cost)
        ws = list(range(lo, hi + 1))
        for _ in range(8):
            if left <= 1.0 or not ws:
                break
            share = left / len(ws)
            nws = []
            for w in ws:
                take = min(share, rem[w])
                alloc[w] += take
                rem[w] -= take
                left -= take
                if rem[w] > 1.0:
                    nws.append(w)
            if len(nws) == len(ws):
                break
            ws = nws
        if left > 1.0:
            alloc[hi] += left  # cram; steps will stretch
    return alloc


def build_program(T, n_full):
    B = n_full // NSHARD  # 16
    ROWS = T * B
    NTF = T * n_full
    nc = bacc.Bacc("TRN2", target_bir_lowering=False, debug=False,
                   num_devices=NCORES)

    inp = {}

    def din(name, shape, dt):
        inp[name] = nc.dram_tensor(name, list(shape), dt, kind="ExternalInput")

    din("xtf0", (128, NTF), F32)
    din("xtf1", (32, NTF), F32)
    din("xto", (160, ROWS), BF16)
    din("whh", (3, 128, 8 * 3072), BF16)
    din("wih12", (2, 16, 128, 3072), BF16)
    din("wih0a", (128, 3072), F32)
    din("wih0b", (32, 3072), F32)
    din("bias0", (1, 3072), F32)
    din("bias12", (2, 1, 3072), BF16)
    din("bhhn", (3, 1, 1024), BF16)
    din("fcwT", (16, 128, NCLS), BF16)
    din("fcb", (1, NCLS), BF16)
    din("gamma", (160, 1), F32)
    din("beta", (160, 1), F32)
    din("ones_bf", (1, 128), BF16)
    din("identrep", (128, 32), BF16)

    out_t = nc.dram_tensor("out", [B, NCLS], F32, kind="ExternalOutput")

    windows = []
    t0 = 0
    while t0 < T:
        windows.append((t0, min(t0 + WSTEPS, T)))
        t0 = min(t0 + WSTEPS, T)
    NW = len(windows)

    def mchunks(r0, r1):
        out = []
        while r0 < r1:
            out.append((r0, min(r0 + 128, r1)))
            r0 = min(r0 + 128, r1)
        return out

    # ---------------- drain plan (pseudo windows = per-layer windows) -----
    npw = 3 * NW

    def pwi(l, wi):
        return l * NW + wi

    pw_steps = [te - ts for _ in range(3) for (ts, te) in windows]
    pw_skip = [0] * npw
    for l in range(3):
        for wi in range(NW):
            i = pwi(l, wi)
            if i == 0:
                pw_skip[i] = 1
            elif l < 2 or wi == 0:
                pw_skip[i] = 4  # real window start (fresh exchange)
            else:
                pw_skip[i] = 0
    caps = [max(0, pw_steps[i] - pw_skip[i]) * GAP_NS for i in range(npw)]

    def gi0_batch_cost(ts, te):
        c = 0.0
        for (r0, r1) in mchunks(ts * B, te * B):
            for (kind, cw) in (("rz", 512), ("n", 256)):
                c += 4 * (2 * MMC(cw) + 250)
        return c

    def gi12_batch_cost(ts, te):
        c = 0.0
        for (r0, r1) in mchunks(ts * B, te * B):
            for (kind, cw) in (("rz", 512), ("n", 256)):
                c += 4 * (16 * MMC(cw) + 300)
        return c

    batches = []
    for wi in range(1, NW):
        ts, te = windows[wi]
        batches.append((gi0_batch_cost(ts, te), 0, pwi(0, wi)))
    for l in range(2):
        for wi in range(NW):
            ts, te = windows[wi]
            batches.append((gi12_batch_cost(ts, te), pwi(l, wi) + 1,
                           pwi(l + 1, wi)))
    alloc = _waterfill(batches, caps)

    tgt_step = np.zeros(3 * T + 1)
    s0 = 0
    for i in range(npw):
        ns, sk = pw_steps[i], pw_skip[i]
        if ns - sk > 0:
            tgt_step[s0 + sk: s0 + ns] = alloc[i] / (ns - sk)
        s0 += ns
    cumtgt = np.cumsum(tgt_step)

    with TileContext(nc) as tc:
        from contextlib import ExitStack
        ctx = ExitStack()
        pers = ctx.enter_context(tc.tile_pool(name="pers", bufs=1))
        gates_pool = ctx.enter_context(
            tc.tile_pool(name="gates", bufs=2, space="PSUM"))
        gi_psum_pool = ctx.enter_context(
            tc.tile_pool(name="gipsum", bufs=3, space="PSUM"))
        dram_pool = ctx.enter_context(
            tc.tile_pool(name="dram", bufs=1, space="DRAM"))
        xch_pool = ctx.enter_context(
            tc.tile_pool(name="xch", bufs=2, space="DRAM"))
        gld_pool = ctx.enter_context(tc.tile_pool(name="gld", bufs=2))
        gicp_pool = ctx.enter_context(tc.tile_pool(name="gicp", bufs=2))

        identrep = pers.tile([128, 32], BF16, tag="identrep")
        nc.sync.dma_start(identrep[:], inp["identrep"][:])
        ones_bf = pers.tile([1, 128], BF16, tag="ones")
        nc.sync.dma_start(ones_bf[:], inp["ones_bf"][:])

        gi_rz = [dram_pool.tile([T, 4, B, 512], BF16, tag=f"girz{l}",
                                name=f"gi_rz{l}") for l in range(3)]
        gi_n = [dram_pool.tile([T, 4, B, 256], BF16, tag=f"gin{l}",
                               name=f"gi_n{l}") for l in range(3)]

        def store_gi(l, g, r0, r1, cc0, cw, gsb):
            mw = r1 - r0
            if cc0 < 512:
                dstt = gi_rz[l][r0 // B: r1 // B, g][:, :, cc0: cc0 + cw]
            else:
                dstt = gi_n[l][r0 // B: r1 // B, g]
            nc.sync.dma_start(dstt, gsb[0:mw, 0:cw])

        # ---------------- phase 0: BN stats ----------------
        stats = []
        with tc.tile_pool(name="ph0s", bufs=1) as ph0s:
            for si, p in ((0, 128), (1, 32)):
                st = ph0s.tile([p, 32], F32, tag=f"st{si}", name=f"st{si}")
                stats.append(st)
                xt = ph0s.tile([p, NTF], F32, tag=f"xt{si}", name=f"xt{si}")
                nc.sync.dma_start(xt[:], inp[f"xtf{si}"][:])
                C = lambda i: st[:, i:i+1]
                nc.vector.tensor_reduce(C(0), xt[:],
                                        axis=mybir.AxisListType.X, op=ALU.add)
                nc.scalar.activation(xt[:], xt[:], AF.Square, accum_out=C(1))
                nc.vector.tensor_scalar_mul(C(2), C(0), 1.0 / NTF)
                nc.vector.tensor_scalar_mul(C(3), C(1), 1.0 / NTF)
                nc.vector.tensor_mul(C(4), C(2), C(2))
                nc.vector.tensor_sub(C(5), C(3), C(4))
                nc.vector.tensor_scalar_add(C(5), C(5), EPS)
                nc.scalar.activation(C(6), C(5), AF.Sqrt)
                nc.vector.reciprocal(C(7), C(6))
                nc.vector.tensor_mul(C(8), C(7), C(7))
                nc.vector.tensor_mul(C(9), C(5), C(8))
                nc.vector.scalar_tensor_tensor(
                    C(10), C(9), -0.5, C(7), op0=ALU.mult, op1=ALU.mult)
                nc.vector.scalar_tensor_tensor(
                    C(11), C(7), 1.5, C(10), op0=ALU.mult, op1=ALU.add)
                nc.sync.dma_start(C(12), inp["gamma"][si*128: si*128+p, :])
                nc.sync.dma_start(C(13), inp["beta"][si*128: si*128+p, :])
                nc.vector.tensor_mul(C(14), C(12), C(11))
                nc.vector.tensor_mul(C(15), C(2), C(14))
                nc.vector.tensor_sub(C(16), C(13), C(15))
            stp = [pers.tile([p_, 32], F32, tag=f"stp{si_}", name=f"stp{si_}")
                   for si_, p_ in ((0, 128), (1, 32))]
            for si in range(2):
                nc.vector.tensor_copy(stp[si][:, 0:17], stats[si][:, 0:17])

        # ---------------- phase 0b: W0 fold + gi0 window 0 ---------
        # ph0k (folded W0 + staged x) stays open for the whole program so
        # the pool stack stays LIFO; the fp32 fold temps live in ph0t,
        # which closes before the scan pools open.
        ph0k = ctx.enter_context(tc.tile_pool(name="ph0k", bufs=1))
        w0ab = ph0k.tile([128, 3072], BF16, tag="w0ab", name="w0ab")
        w0bb = ph0k.tile([32, 3072], BF16, tag="w0bb", name="w0bb")
        xto_a = ph0k.tile([128, ROWS], BF16, tag="xtoa", name="xtoa")
        nc.sync.dma_start(xto_a[:], inp["xto"][0:128, :])
        xto_b = ph0k.tile([32, ROWS], BF16, tag="xtob", name="xtob")
        nc.sync.dma_start(xto_b[:], inp["xto"][128:160, :])
        KB = CIN + 1 - 128  # 23 rows incl folded bias row

        with tc.tile_pool(name="ph0t", bufs=1) as ph0t:
            w0a = ph0t.tile([128, 3072], F32, tag="w0a", name="w0a")
            nc.sync.dma_start(w0a[:], inp["wih0a"][:])
            w0b = ph0t.tile([32, 3072], F32, tag="w0b", name="w0b")
            nc.sync.dma_start(w0b[:], inp["wih0b"][:])
            bias0 = ph0t.tile([1, 3072], F32, tag="bias0t", name="bias0t")
            nc.sync.dma_start(bias0[:], inp["bias0"][:])

            w0rows = [(w0a, 128, stp[0]), (w0b, CIN - 128, stp[1])]
            for n in range(6):
                bps = gi_psum_pool.tile([128, 512], F32, tag="gips",
                                        name=f"bps{n}")
                for ki, (w0, kp, st) in enumerate(w0rows):
                    nc.tensor.matmul(
                        bps[0:1, 0:512], st[0:kp, 16:17],
                        w0[0:kp, n * 512: (n + 1) * 512],
                        start=(ki == 0), stop=(ki == 1))
                nc.vector.tensor_add(bias0[:, n * 512: (n + 1) * 512],
                                     bps[0:1, 0:512],
                                     bias0[:, n * 512: (n + 1) * 512])
            for w0, kp, st in w0rows:
                nc.vector.tensor_scalar_mul(w0[0:kp, :], w0[0:kp, :],
                                            st[0:kp, 14:15])
            nc.sync.dma_start(w0b[CIN - 128: CIN - 128 + 1, :], bias0[:])
            nc.vector.tensor_copy(w0ab[:], w0a[:])
            nc.vector.tensor_copy(w0bb[0:KB, :], w0b[0:KB, :])

        scan_pool = ctx.enter_context(tc.tile_pool(name="scan", bufs=1))
        owin_pool = ctx.enter_context(tc.tile_pool(name="owin", bufs=2))
        og_pool = ctx.enter_context(tc.tile_pool(name="og", bufs=2))
        wih_pool = ctx.enter_context(tc.tile_pool(name="wih", bufs=2))

        copyout_q = []

        def gi0_job(g, kind, cc0, cw, r0, r1):
            mw = r1 - r0
            gps = gi_psum_pool.tile([128, 512], F32, tag="gips",
                                    name=f"g0ps{r0}_{g}_{kind}")
            nc.tensor.matmul(
                gps[0:mw, 0:cw], xto_a[:, r0:r1],
                w0ab[:, g * 768 + cc0: g * 768 + cc0 + cw],
                start=True, stop=False)
            nc.tensor.matmul(
                gps[0:mw, 0:cw], xto_b[0:KB, r0:r1],
                w0bb[0:KB, g * 768 + cc0: g * 768 + cc0 + cw],
                start=False, stop=True)

            def copy_out():
                gsb = gicp_pool.tile([128, cw], BF16, tag=f"gisb_{kind}",
                                     name=f"g0sb{r0}_{g}_{kind}")
                nc.scalar.copy(gsb[0:mw, :], gps[0:mw, 0:cw])
                store_gi(0, g, r0, r1, cc0, cw, gsb)
            copyout_q.append(copy_out)

        # upfront: gi0 for window 0 rows
        up_hi = min(windows[0][1] * B, ROWS)
        for (r0, r1) in mchunks(0, up_hi):
            for g in range(4):
                for (kind, cc0, cw) in (("rz", 0, 512), ("n", 512, 256)):
                    gi0_job(g, kind, cc0, cw, r0, r1)
                    while copyout_q:
                        copyout_q.pop(0)()

        # deferred gi0 jobs (drained in layer-0 scan gaps)
        pending = []  # (cost, min_gstep, fn, mid, seq)
        seq_ctr = [0]
        batch_seq = {}  # (gi_layer, window_idx) -> last seq of that batch

        def bump_seq(key):
            seq_ctr[0] += 1
            batch_seq[key] = seq_ctr[0]
            return seq_ctr[0]

        for wi in range(1, NW):
            ts, te = windows[wi]
            sq = bump_seq((0, wi))
            for (r0, r1) in mchunks(ts * B, te * B):
                for g in range(4):
                    for (kind, cc0, cw) in (("rz", 0, 512), ("n", 512, 256)):
                        pending.append((2 * MMC(cw) + 250, 0,
                                        (lambda g_=g, k_=kind, c_=cc0,
                                         w_=cw, a_=r0, b_=r1:
                                         gi0_job(g_, k_, c_, w_, a_, b_)),
                                        False, sq))

        # ---------------- scan setup ----------------
        outT_last = pers.tile([128, 256], BF16, tag="outTlast",
                              name="outT_last")

        def exchange_window(l, ts, te, owin):
            """AllGather packed own outT window with pair core."""
            wsz = (te - ts) * B
            tagsfx = f"{l}_{ts}"
            xin = xch_pool.tile([128, 8 * wsz], BF16, tag="xin",
                                name=f"xin_{tagsfx}")
            # pack k-major (j, s, b16): one DMA per j keeps APs <= 3 dims
            ow_v = owin.rearrange("p (s j b) -> p s j b", j=8, b=32)
            for j in range(8):
                nc.sync.dma_start(
                    xin[:, j * wsz: (j + 1) * wsz].rearrange(
                        "p (s b) -> p s b", b=16),
                    ow_v[:, 0: te - ts, j, 0:16])
            xout = xch_pool.tile([2, 128, 8 * wsz], BF16, tag="xout",
                                 name=f"xout_{tagsfx}")
            nc.gpsimd.collective_compute(
                "AllGather", ALU.bypass, replica_groups=PAIRS,
                ins=[xin[:].opt()], outs=[xout[:].opt()])
            og = [og_pool.tile([128, 8 * wsz], BF16, tag=f"ogd{d}",
                               name=f"og{d}_{tagsfx}") for d in range(2)]
            for d in range(2):
                nc.sync.dma_start(og[d][:, 0: 8 * wsz], xout[d])
            return og

        def exchange_last(owin2, slot):
            xin = xch_pool.tile([128, 128], BF16, tag="xinL", name="xin_last")
            src = owin2[:, slot * 256: (slot + 1) * 256].rearrange(
                "p (j b) -> p j b", j=8)[:, :, 0:16]
            nc.sync.dma_start(
                xin.rearrange("p (j b) -> p j b", j=8), src)
            xout = xch_pool.tile([2, 128, 128], BF16, tag="xoutL",
                                 name="xout_last")
            nc.gpsimd.collective_compute(
                "AllGather", ALU.bypass, replica_groups=PAIRS,
                ins=[xin[:].opt()], outs=[xout[:].opt()])
            for d in range(2):
                nc.sync.dma_start(outT_last[:, d * 128: (d + 1) * 128],
                                  xout[d])

        def gi_jobs(l, ts, te, og, avail_gstep):
            """Sub-jobs for the layer-(l+1) gi GEMM of a window."""
            nsw = te - ts
            groups = [(g, kind, cc0, cw)
                      for g in range(4)
                      for (kind, cc0, cw) in (("rz", 0, 512),
                                              ("n", 512, 256))]
            tiles = {}
            psums = {}

            def load_job(gi_idx):
                g, kind, cc0, cw = groups[gi_idx]
                col = g * 768 + cc0
                wt = wih_pool.tile([128, 16 * cw], BF16, tag="wihT",
                                   name=f"wt_{l}{ts}{g}{kind}")
                for k in range(16):
                    nc.sync.dma_start(
                        wt[:, k * cw: (k + 1) * cw],
                        inp["wih12"][l, k, :, col: col + cw])
                bt = wih_pool.tile([1, cw], BF16, tag="biasT",
                                   name=f"bt_{l}{ts}{g}{kind}")
                nc.sync.dma_start(bt[:], inp["bias12"][l, :, col: col + cw])
                tiles[gi_idx] = (wt, bt)

            def mm_sub(gi_idx, r0, r1, k0, k1, last):
                g, kind, cc0, cw = groups[gi_idx]
                wt, bt = tiles[gi_idx]
                mw = r1 - r0
                q0 = r0 - ts * B
                wsz = (te - ts) * B
                if k0 == 0:
                    psums[(gi_idx, r0)] = gi_psum_pool.tile(
                        [128, 512], F32, tag="gips",
                        name=f"gp_{l}{ts}{g}{kind}{r0}")
                gps = psums[(gi_idx, r0)]
                for k in range(k0, k1):
                    dsrc, kk = k // 8, k % 8
                    lhsT = og[dsrc][:, kk * wsz + q0: kk * wsz + q0 + mw]
                    nc.tensor.matmul(
                        gps[0:mw, 0:cw], lhsT,
                        wt[:, k * cw: (k + 1) * cw],
                        start=(k == 0), stop=False)
                if last:
                    nc.tensor.matmul(
                        gps[0:mw, 0:cw], ones_bf[:, 0:mw], bt[:],
                        start=False, stop=True)

                    def copy_out():
                        if DEBUG_EMIT and l == 1:
                            print(f"EMIT store l2 g={g} kind={kind} "
                                  f"r0={r0} gstep={gstep[0]}")
                        gsb = gicp_pool.tile(
                            [128, cw], BF16, tag=f"gisb_{kind}",
                            name=f"gs_{l}{ts}{g}{kind}{r0}")
                        nc.scalar.copy(gsb[0:mw, :], gps[0:mw, 0:cw])
                        store_gi(l + 1, g, r0, r1, cc0, cw, gsb)
                        del psums[(gi_idx, r0)]
                    copyout_q.append(copy_out)

            jobs = []
            rcs = mchunks(ts * B, te * B)
            mg = avail_gstep + 4
            sq = bump_seq((l + 1, ts // WSTEPS))
            jobs.append((150, mg, lambda: load_job(0), False, sq))
            for gi_idx in range(len(groups)):
                g, kind, cc0, cw = groups[gi_idx]
                if gi_idx + 1 < len(groups):
                    jobs.append((150, mg, lambda i=gi_idx + 1: load_job(i),
                                 False, sq))
                ksp = ((0, 4), (4, 8), (8, 12), (12, 16)) if kind == "rz" \
                    else ((0, 8), (8, 16))
                for (r0, r1) in rcs:
                    for (k0, k1) in ksp:
                        cost = (k1 - k0) * MMC(cw) + (300 if k1 == 16 else 0)
                        jobs.append((cost, mg,
                                     lambda i=gi_idx, a=r0, b=r1, x=k0, y=k1,
                                     z=(k1 == 16): mm_sub(i, a, b, x, y, z),
                                     k0 > 0, sq))
            return jobs

        # ---------------- layers ----------------
        drained = [0.0]
        gstep = [0]

        def drain_through(key):
            """Emit all pending jobs up to and including batch `key` so the
            gi DRAM rows a chunk-load reads are stored first."""
            tgt = batch_seq.get(key)
            if tgt is None:
                return
            while pending and pending[0][4] <= tgt:
                e = pending.pop(0)
                e[2]()
                drained[0] += e[0]
            while copyout_q:
                copyout_q.pop(0)()

        h_elem = [scan_pool.tile([128, 256], BF16, tag=f"h_{par}",
                                 name=f"h_{par}") for par in range(2)]
        zlhs = scan_pool.tile([128, B], BF16, tag="zlhs", name="zlhs")
        nc.vector.memset(zlhs[:], 0.0)
        scr = scan_pool.tile([128, 2048], BF16, tag="scr", name="scr")
        owin2 = scan_pool.tile([128, 512], BF16, tag="owin2", name="owin2")

        for l in range(3):
            whh_sb = scan_pool.tile([128, 8 * 3072], BF16, tag="whh_sb",
                                    name=f"whh_sb{l}")
            for q in range(4):
                nc.sync.dma_start(whh_sb[:, q * 6144: (q + 1) * 6144],
                                  inp["whh"][l][:, q * 6144: (q + 1) * 6144])
            bhhn_sb = scan_pool.tile([1, 1024], BF16, tag="bhhn_sb",
                                     name=f"bhhn_sb{l}")
            nc.sync.dma_start(bhhn_sb[:], inp["bhhn"][l])

            nc.vector.memset(h_elem[0][:], 0.0)

            windows_l = windows if l < 2 else [(0, T)]

            chunk_tiles = {}

            def load_chunk(t0, te):
                if DEBUG_EMIT and l == 2:
                    print(f"EMIT load l2 t0={t0} gstep={gstep[0]}")
                tS = min(SCH, te - t0)
                drain_through((l, (t0 + tS - 1) // WSTEPS))
                grz = gld_pool.tile([128, SCH * 512], BF16, tag="grz",
                                    name=f"grz_{l}_{t0}")
                gst = gld_pool.tile([128, SCH * 256], BF16, tag="gst",
                                    name=f"gst_{l}_{t0}")
                if SIM_SAFE:
                    nc.vector.memset(grz[:], 0.0)
                    nc.vector.memset(gst[:], 0.0)
                for g in range(4):
                    nc.sync.dma_start(
                        grz[32*g: 32*g + B, 0: tS * 512],
                        gi_rz[l][t0: t0 + tS, g].rearrange(
                            "s b c -> b s c"))
                    nc.sync.dma_start(
                        gst[32*g: 32*g + B, 0: tS * 256],
                        gi_n[l][t0: t0 + tS, g].rearrange(
                            "s b c -> b s c"))
                chunk_tiles[t0] = (grz, gst)
                return grz, gst

            def emit_pre(t, ts):
                """ident-MM gi_rz preload + n-bias MM for step t (PSUM)."""
                so = (t - ts) % SCH
                grz, _ = chunk_tiles[ts + ((t - ts) // SCH) * SCH]
                gp = gates_pool.tile([128, 768], F32, tag="gp",
                                     name=f"gp_{l}_{t}")
                for g in range(4):
                    nc.tensor.matmul(
                        gp[32*g: 32*g + 32, 0:512],
                        identrep[32*g: 32*g + 16, 0:32],
                        grz[32*g: 32*g + 16, so * 512: (so + 1) * 512],
                        start=True, stop=False,
                        skip_group_check=True,
                        tile_position=(32 * g, 32 * g))
                for g in range(4):
                    nc.tensor.matmul(
                        gp[32*g: 32*g + 32, 512:768],
                        ones_bf[:, 0:32],
                        bhhn_sb[:, g * 256: (g+1) * 256],
                        start=True, stop=False,
                        skip_group_check=True,
                        tile_position=(0, 32 * g))
                return gp

            owin_prev, nsteps_prev = None, 0
            for (ts, te) in windows_l:
                nsteps = te - ts
                if l < 2:
                    owin = owin_pool.tile([128, nsteps * 256], BF16,
                                          tag="owin", name=f"owin_{l}_{ts}")
                else:
                    owin = owin2
                # chunk prefetch: first two chunks of this window
                load_chunk(ts, te)
                if ts + SCH < te:
                    load_chunk(ts + SCH, te)
                gp_next = emit_pre(ts, ts)

                for t in range(ts, te):
                    so = (t - ts) % SCH
                    gp = gp_next
                    grz_c, gst_c = chunk_tiles[ts + ((t - ts) // SCH) * SCH]

                    def lhsT_for(j):
                        if t == 0:
                            return zlhs[:, 0:B]
                        if l < 2:
                            if t == ts:
                                return owin_prev[
                                    :, (nsteps_prev - 1) * 256 + j * 32:
                                    (nsteps_prev - 1) * 256 + j * 32 + 16]
                            return owin[:, (t - 1 - ts) * 256 + j * 32:
                                        (t - 1 - ts) * 256 + j * 32 + 16]
                        return owin2[:, ((t - 1) % 2) * 256 + j * 32:
                                     ((t - 1) % 2) * 256 + j * 32 + 16]

                    for k in range(8):
                        lhsT = lhsT_for(k)
                        for (c0, cw) in ((0, 512), (512, 256)):
                            for g in range(4):
                                nc.tensor.matmul(
                                    gp[32*g: 32*g + B, c0: c0 + cw],
                                    lhsT,
                                    whh_sb[:, k * 3072 + g * 768 + c0:
                                           k * 3072 + g * 768 + c0 + cw],
                                    start=False,
                                    stop=(k == 7),
                                    skip_group_check=True,
                                    tile_position=(0, 32 * g))

                    # issue next chunk prefetch at each chunk boundary
                    # (gld bufs=2: the reused buf's reads were all emitted
                    # during the chunk before last)
                    if so == 0 and t > ts and t + SCH < te:
                        load_chunk(t + SCH, te)
                    # preload for next step (runs in this step's PE gap)
                    if t + 1 < te:
                        gp_next = emit_pre(t + 1, ts)

                    # drain gi jobs into the PE gap per the global plan
                    nemit = 0
                    allowed = cumtgt[gstep[0]]
                    while (pending and drained[0] < allowed
                           and pending[0][1] <= gstep[0]):
                        cost, _, fn, _mid, _sq = pending.pop(0)
                        fn()
                        drained[0] += cost
                        nemit += 1
                    # finish an in-flight job's remaining sub-jobs before
                    # anything else can recycle its PSUM accumulator
                    while (pending and pending[0][3]
                           and pending[0][1] <= gstep[0]):
                        cost, _, fn, _mid, _sq = pending.pop(0)
                        fn()
                        drained[0] += cost
                        nemit += 1
                    if nemit == 0 and t - ts >= 2:
                        wps = gi_psum_pool.tile([128, 512], F32, tag="gips",
                                                name=f"warm_{l}_{t}")
                        for wq in range(3):
                            nc.tensor.matmul(
                                wps[0:B, 0:512], zlhs[:, 0:B],
                                whh_sb[:, wq * 512: (wq + 1) * 512],
                                start=(wq == 0), stop=(wq == 2))

                    # ---- activation chain, split into cj halves ----
                    h_prev = h_elem[t % 2]
                    h_new = h_elem[(t + 1) % 2]
                    rz = scr[:, 0:512]
                    omz = scr[:, 512:768]
                    t1 = scr[:, 768:1024]
                    t2 = scr[:, 1024:1280]
                    nt = scr[:, 1280:1536]
                    zh = scr[:, 1536:1792]
                    u = scr[:, 1792:2048]

                    gp_rz = gp[:, 0:512].rearrange(
                        "p (two c) -> p two c", two=2)
                    rz_v = rz.rearrange("p (two c) -> p two c", two=2)

                    def half(h0):
                        h1 = h0 + 128
                        nc.scalar.activation(
                            rz_v[:, :, h0:h1], gp_rz[:, :, h0:h1],
                            AF.Sigmoid)
                        nc.vector.tensor_mul(
                            t1[:, h0:h1], rz[:, h0:h1],
                            gp[:, 512 + h0: 512 + h1])
                        nc.vector.tensor_add(
                            t2[:, h0:h1], t1[:, h0:h1],
                            gst_c[:, so * 256 + h0: so * 256 + h1])
                        nc.scalar.activation(nt[:, h0:h1], t2[:, h0:h1],
                                             AF.Tanh)
                        nc.vector.tensor_scalar(
                            omz[:, h0:h1], rz[:, 256 + h0: 256 + h1],
                            -1.0, 1.0, op0=ALU.mult, op1=ALU.add)
                        nc.vector.tensor_mul(
                            zh[:, h0:h1], rz[:, 256 + h0: 256 + h1],
                            h_prev[:, h0:h1])

                    def tail(h0):
                        h1 = h0 + 128
                        nc.vector.tensor_mul(u[:, h0:h1], omz[:, h0:h1],
                                             nt[:, h0:h1])
                        nc.vector.tensor_add(h_new[:, h0:h1], u[:, h0:h1],
                                             zh[:, h0:h1])

                    half(0)
                    half(128)
                    tail(0)
                    tail(128)

                    if l < 2:
                        dst = owin[:, (t - ts) * 256: (t - ts + 1) * 256]
                    else:
                        dst = owin2[:, (t % 2) * 256: (t % 2 + 1) * 256]
                    nc.vector.transpose(dst, h_new[:])

                    while copyout_q:
                        copyout_q.pop(0)()
                    gstep[0] += 1

                # end of window
                if l < 2:
                    og = exchange_window(l, ts, te, owin)
                    pending.extend(gi_jobs(l, ts, te, og, gstep[0]))
                owin_prev, nsteps_prev = owin, nsteps

        # flush any gi jobs not drained during step gaps
        while pending:
            pending.pop(0)[2]()  # noqa
        while copyout_q:
            copyout_q.pop(0)()

        exchange_last(owin2, (T - 1) % 2)

        # ---------------- FC ----------------
        fcw = pers.tile([128, 16 * NCLS], BF16, tag="fcw")
        for k in range(16):
            nc.sync.dma_start(fcw[:, k * NCLS: (k + 1) * NCLS],
                              inp["fcwT"][k])
        fcb = pers.tile([1, NCLS], BF16, tag="fcb")
        nc.sync.dma_start(fcb[:], inp["fcb"][:])
        fps = gi_psum_pool.tile([128, 512], F32, tag="gips", name="fps")
        for k in range(16):
            nc.tensor.matmul(
                fps[0:B, 0:NCLS],
                outT_last[:, k * 16: k * 16 + 16],
                fcw[:, k * NCLS: (k + 1) * NCLS],
                start=(k == 0), stop=False)
        nc.tensor.matmul(fps[0:B, 0:NCLS], ones_bf[:, 0:B], fcb[:],
                         start=False, stop=True)
        fout = gicp_pool.tile([B, NCLS], F32, tag="fout")
        nc.vector.tensor_copy(fout[:], fps[0:B, 0:NCLS])
        nc.sync.dma_start(out_t[:], fout[:])

        ctx.close()

    nc.compile()
    return nc


_cache = {}


def kernel(**inputs):
    T = inputs["x"].shape[1]
    n_full = inputs["x"].shape[0]
    key = ("prog", T, n_full)
    if key not in _cache:
        _cache[key] = build_program(T, n_full)
    nc = _cache[key]
    per_core = host_prep(inputs, T, n_full)
    res = run_bass_kernel_spmd(nc, per_core, core_ids=list(range(NCORES)))
    out = np.concatenate([res.results[2 * s]["out"] for s in range(NSHARD)],
                         axis=0)
    return np.ascontiguousarray(out.astype(np.float32))


# revision 24
# speedup vs baseline: 1.1235x; 1.0268x over previous
"""Trainium2 Bass kernel for a 3-layer bidirectional GRU classifier.

Sharding: 8 cores = 4 batch shards (16 samples) x 2 directions.  Each core
runs only its own direction's recurrence and computes its own direction's
input projections (gi) for the next layer.  Direction pairs exchange
transposed hidden outputs per window via pairwise AllGather so the gi GEMM
can contract over the full 2H=2048 input.  Both cores of a pair compute the
same FC output for their shard; kernel() reads cores 0,2,4,6.

h^T layout (owin): produced per step by one DVE StreamTranspose of
h_new [128=4a x 32(b<16), 256=cj] -> owin[32a+m, j*32+b] = h[b, a*256+j*32+m]
so rec lhsT k-chunk j is owin[:, s*256 + j*32 : +16].  Weights are permuted
host-side to match.
"""

import os
import sys

for _p in ("/opt/trn_rl_repo", "/root/.axon_site/_ro/trn_rl_repo"):
    if os.path.isdir(_p) and _p not in sys.path:
        sys.path.append(_p)

import numpy as np
import ml_dtypes

import concourse.bacc as bacc
import concourse.mybir as mybir
from concourse.tile import TileContext
from concourse.bass_utils import run_bass_kernel_spmd

BF16 = mybir.dt.bfloat16
F32 = mybir.dt.float32
AF = mybir.ActivationFunctionType
ALU = mybir.AluOpType

NCORES = 8
NSHARD = 4
H = 1024
G3 = 3 * H
CIN = 150
NCLS = 60
EPS = 1e-5
PAIRS = [[0, 1], [2, 3], [4, 5], [6, 7]]

WSTEPS = 30
SCH = 8
GAP_NS = 2600.0
DEBUG_EMIT = False
SIM_SAFE = False  # set True for MultiCoreSim runs (init junk lanes)


def MMC(cw):
    return cw / 2.4 + 50.0


def _perm3h(v):
    """Reorder a [..., 3072] gate-major vector to (a, gate, cj) column order."""
    a = v.reshape(*v.shape[:-1], 3, 4, 256)
    a = np.moveaxis(a, -3, -2)  # [..., 4, 3, 256]
    return a.reshape(*v.shape[:-1], G3)


def host_prep(inputs, T, n_full):
    B = n_full // NSHARD  # 16 samples per shard
    x = np.asarray(inputs["x"], np.float32)
    gamma = np.asarray(inputs["bn_gamma"], np.float32)
    beta = np.asarray(inputs["bn_beta"], np.float32)
    w_ih0 = np.asarray(inputs["w_ih0"], np.float32)
    w_hh0 = np.asarray(inputs["w_hh0"], np.float32)
    b_ih0 = np.asarray(inputs["b_ih0"], np.float32)
    b_hh0 = np.asarray(inputs["b_hh0"], np.float32)
    w_ih = np.asarray(inputs["w_ih"], np.float32)
    w_hh = np.asarray(inputs["w_hh"], np.float32)
    b_ih = np.asarray(inputs["b_ih"], np.float32)
    b_hh = np.asarray(inputs["b_hh"], np.float32)
    fc_w = np.asarray(inputs["fc_w"], np.float32)
    fc_b = np.asarray(inputs["fc_b"], np.float32)

    shared = {}
    NTF = T * n_full

    xTf = np.ascontiguousarray(x.transpose(2, 1, 0).reshape(CIN, NTF))
    shared["xtf0"] = np.ascontiguousarray(xTf[:128])
    xtf1 = np.zeros((32, NTF), np.float32)
    xtf1[: CIN - 128] = xTf[128:]
    shared["xtf1"] = xtf1

    # W_hh^T per (layer, dir): [128=ai*32+m, 8j*3072] with in-dim
    # d = ai*256 + j*32 + m, cols (j, a', gate, cj)
    whh_all = np.stack([w_hh0, w_hh[0], w_hh[1]])  # [3,2,3072,1024]
    t = whh_all.reshape(3, 2, 3, 4, 256, 4, 8, 32)  # l d g3 a' cj ai j m
    t = np.transpose(t, (0, 1, 5, 7, 6, 3, 2, 4))  # l d ai m j a' g3 cj
    whh_p = np.ascontiguousarray(t.reshape(3, 2, 128, 8 * 3072)).astype(
        ml_dtypes.bfloat16)

    # W_ih layers 1,2: [l, k=(dsrc*8+j), q=ai*32+m, dir, col(a',gate,cj)]
    t = w_ih.reshape(2, 2, 3, 4, 256, 2, 4, 8, 32)  # l d g3 a' cj ds ai j m
    t = np.transpose(t, (0, 5, 7, 6, 8, 1, 3, 2, 4))  # l ds j ai m d a' g3 cj
    wih12_p = t.reshape(2, 16, 128, 2, 3072).astype(ml_dtypes.bfloat16)

    # W_ih0^T [150->(128,32), 2, 3072] (contraction over CIN: unaffected)
    t0 = w_ih0.reshape(2, 3, 4, 256, CIN)
    t0 = np.transpose(t0, (4, 0, 2, 1, 3)).reshape(CIN, 2, 3072)

    mask_rz = np.zeros(G3, np.float32)
    mask_rz[: 2 * H] = 1.0
    bias0_d = [_perm3h(b_ih0[d] + b_hh0[d] * mask_rz) for d in range(2)]
    b12_d = [[_perm3h(b_ih[li, d] + b_hh[li, d] * mask_rz) for li in range(2)]
             for d in range(2)]

    bhh_all = np.stack([b_hh0, b_hh[0], b_hh[1]])[:, :, 2 * H:]  # [3,2,1024]

    # FC: fcwT[k=(d*8+j)][q=ai*32+m, c]
    fw = fc_w.reshape(NCLS, 2, 4, 8, 32)  # c d ai j m
    fw = np.transpose(fw, (1, 3, 2, 4, 0))  # d j ai m c
    shared["fcwT"] = np.ascontiguousarray(
        fw.reshape(16, 128, NCLS).astype(ml_dtypes.bfloat16))
    shared["fcb"] = fc_b.reshape(1, NCLS).astype(ml_dtypes.bfloat16)

    shared["gamma"] = np.concatenate(
        [gamma, np.zeros(10, np.float32)]).reshape(160, 1)
    shared["beta"] = np.concatenate(
        [beta, np.zeros(10, np.float32)]).reshape(160, 1)

    shared["ones_bf"] = np.ones((1, 128), ml_dtypes.bfloat16)
    idr = np.zeros((128, 32), np.float32)
    for g in range(4):
        for i in range(16):
            idr[32 * g + i, i] = 1.0
            idr[32 * g + i, 16 + i] = 1.0
    shared["identrep"] = idr.astype(ml_dtypes.bfloat16)

    per_core = []
    for c in range(NCORES):
        s, d = c // 2, c % 2
        m = dict(shared)
        m["whh"] = np.ascontiguousarray(whh_p[:, d])  # [3, 128, 24576]
        m["wih12"] = np.ascontiguousarray(wih12_p[:, :, :, d])  # [2,16,128,3072]
        wd = np.ascontiguousarray(t0[:, d])  # [150, 3072]
        m["wih0a"] = np.ascontiguousarray(wd[:128])
        w0b = np.zeros((32, 3072), np.float32)
        w0b[: CIN - 128] = wd[128:]
        m["wih0b"] = w0b
        m["bias0"] = bias0_d[d].reshape(1, 3072)
        m["bhhn"] = np.ascontiguousarray(
            bhh_all[:, d].reshape(3, 1, 1024).astype(ml_dtypes.bfloat16))
        m["bias12"] = np.stack(b12_d[d]).reshape(2, 1, 3072).astype(
            ml_dtypes.bfloat16)

        xo = x[s * B: (s + 1) * B]
        xT = xo.transpose(2, 1, 0).reshape(CIN, T * B)
        aug = np.zeros((160, T * B), np.float32)
        aug[:CIN] = xT
        aug[CIN] = 1.0
        m["xto"] = aug.astype(ml_dtypes.bfloat16)
        per_core.append(m)
    return per_core


def _waterfill(batches, caps):
    """batches: list of (cost, avail, dl). Returns per-window alloc list."""
    n = len(caps)
    alloc = [0.0] * n
    rem = list(caps)
    for cost, avail, dl in batches:
        lo = max(0, min(avail, n - 1))
        hi = max(lo, min(dl - 1, n - 1))
        left = float(cost)
        ws = list(range(lo, hi + 1))
        for _ in range(8):
            if left <= 1.0 or not ws:
                break
            share = left / len(ws)
            nws = []
            for w in ws:
                take = min(share, rem[w])
                alloc[w] += take
                rem[w] -= take
                left -= take
                if rem[w] > 1.0:
                    nws.append(w)
            if len(nws) == len(ws):
                break
            ws = nws
        if left > 1.0:
            alloc[hi] += left  # cram; steps will stretch
    return alloc


def build_program(T, n_full):
    B = n_full // NSHARD  # 16
    ROWS = T * B
    NTF = T * n_full
    nc = bacc.Bacc("TRN2", target_bir_lowering=False, debug=False,
                   num_devices=NCORES)

    inp = {}

    def din(name, shape, dt):
        inp[name] = nc.dram_tensor(name, list(shape), dt, kind="ExternalInput")

    din("xtf0", (128, NTF), F32)
    din("xtf1", (32, NTF), F32)
    din("xto", (160, ROWS), BF16)
    din("whh", (3, 128, 8 * 3072), BF16)
    din("wih12", (2, 16, 128, 3072), BF16)
    din("wih0a", (128, 3072), F32)
    din("wih0b", (32, 3072), F32)
    din("bias0", (1, 3072), F32)
    din("bias12", (2, 1, 3072), BF16)
    din("bhhn", (3, 1, 1024), BF16)
    din("fcwT", (16, 128, NCLS), BF16)
    din("fcb", (1, NCLS), BF16)
    din("gamma", (160, 1), F32)
    din("beta", (160, 1), F32)
    din("ones_bf", (1, 128), BF16)
    din("identrep", (128, 32), BF16)

    out_t = nc.dram_tensor("out", [B, NCLS], F32, kind="ExternalOutput")

    windows = []
    t0 = 0
    while t0 < T:
        windows.append((t0, min(t0 + WSTEPS, T)))
        t0 = min(t0 + WSTEPS, T)
    NW = len(windows)

    def mchunks(r0, r1):
        out = []
        while r0 < r1:
            out.append((r0, min(r0 + 128, r1)))
            r0 = min(r0 + 128, r1)
        return out

    # ---------------- drain plan (pseudo windows = per-layer windows) -----
    npw = 3 * NW

    def pwi(l, wi):
        return l * NW + wi

    pw_steps = [te - ts for _ in range(3) for (ts, te) in windows]
    pw_skip = [0] * npw
    for l in range(3):
        for wi in range(NW):
            i = pwi(l, wi)
            if i == 0:
                pw_skip[i] = 1
            elif l < 2 or wi == 0:
                pw_skip[i] = 4  # real window start (fresh exchange)
            else:
                pw_skip[i] = 0
    caps = [max(0, pw_steps[i] - pw_skip[i]) * GAP_NS for i in range(npw)]

    def gi0_batch_cost(ts, te):
        c = 0.0
        for (r0, r1) in mchunks(ts * B, te * B):
            for (kind, cw) in (("rz", 512), ("n", 256)):
                c += 4 * (2 * MMC(cw) + 250)
        return c

    def gi12_batch_cost(ts, te):
        c = 0.0
        for (r0, r1) in mchunks(ts * B, te * B):
            for (kind, cw) in (("rz", 512), ("n", 256)):
                c += 4 * (16 * MMC(cw) + 300)
        return c

    batches = []
    for wi in range(1, NW):
        ts, te = windows[wi]
        batches.append((gi0_batch_cost(ts, te), 0, pwi(0, wi)))
    for l in range(2):
        for wi in range(NW):
            ts, te = windows[wi]
            batches.append((gi12_batch_cost(ts, te), pwi(l, wi) + 1,
                           pwi(l + 1, wi)))
    alloc = _waterfill(batches, caps)

    tgt_step = np.zeros(3 * T + 1)
    s0 = 0
    for i in range(npw):
        ns, sk = pw_steps[i], pw_skip[i]
        if ns - sk > 0:
            tgt_step[s0 + sk: s0 + ns] = alloc[i] / (ns - sk)
        s0 += ns
    cumtgt = np.cumsum(tgt_step)

    with TileContext(nc) as tc:
        from contextlib import ExitStack
        ctx = ExitStack()
        pers = ctx.enter_context(tc.tile_pool(name="pers", bufs=1))
        gates_pool = ctx.enter_context(
            tc.tile_pool(name="gates", bufs=2, space="PSUM"))
        gi_psum_pool = ctx.enter_context(
            tc.tile_pool(name="gipsum", bufs=3, space="PSUM"))
        dram_pool = ctx.enter_context(
            tc.tile_pool(name="dram", bufs=1, space="DRAM"))
        xch_pool = ctx.enter_context(
            tc.tile_pool(name="xch", bufs=2, space="DRAM"))
        gld_pool = ctx.enter_context(tc.tile_pool(name="gld", bufs=2))
        gicp_pool = ctx.enter_context(tc.tile_pool(name="gicp", bufs=2))

        identrep = pers.tile([128, 32], BF16, tag="identrep")
        nc.sync.dma_start(identrep[:], inp["identrep"][:])
        ones_bf = pers.tile([1, 128], BF16, tag="ones")
        nc.sync.dma_start(ones_bf[:], inp["ones_bf"][:])

        gi_rz = [dram_pool.tile([T, 4, B, 512], BF16, tag=f"girz{l}",
                                name=f"gi_rz{l}") for l in range(3)]
        gi_n = [dram_pool.tile([T, 4, B, 256], BF16, tag=f"gin{l}",
                               name=f"gi_n{l}") for l in range(3)]

        def store_gi(l, g, r0, r1, cc0, cw, gsb):
            mw = r1 - r0
            if cc0 < 512:
                dstt = gi_rz[l][r0 // B: r1 // B, g][:, :, cc0: cc0 + cw]
            else:
                dstt = gi_n[l][r0 // B: r1 // B, g]
            nc.sync.dma_start(dstt, gsb[0:mw, 0:cw])

        # ---------------- phase 0: BN stats ----------------
        stats = []
        with tc.tile_pool(name="ph0s", bufs=1) as ph0s:
            for si, p in ((0, 128), (1, 32)):
                st = ph0s.tile([p, 32], F32, tag=f"st{si}", name=f"st{si}")
                stats.append(st)
                xt = ph0s.tile([p, NTF], F32, tag=f"xt{si}", name=f"xt{si}")
                nc.sync.dma_start(xt[:], inp[f"xtf{si}"][:])
                C = lambda i: st[:, i:i+1]
                nc.vector.tensor_reduce(C(0), xt[:],
                                        axis=mybir.AxisListType.X, op=ALU.add)
                nc.scalar.activation(xt[:], xt[:], AF.Square, accum_out=C(1))
                nc.vector.tensor_scalar_mul(C(2), C(0), 1.0 / NTF)
                nc.vector.tensor_scalar_mul(C(3), C(1), 1.0 / NTF)
                nc.vector.tensor_mul(C(4), C(2), C(2))
                nc.vector.tensor_sub(C(5), C(3), C(4))
                nc.vector.tensor_scalar_add(C(5), C(5), EPS)
                nc.scalar.activation(C(6), C(5), AF.Sqrt)
                nc.vector.reciprocal(C(7), C(6))
                nc.vector.tensor_mul(C(8), C(7), C(7))
                nc.vector.tensor_mul(C(9), C(5), C(8))
                nc.vector.scalar_tensor_tensor(
                    C(10), C(9), -0.5, C(7), op0=ALU.mult, op1=ALU.mult)
                nc.vector.scalar_tensor_tensor(
                    C(11), C(7), 1.5, C(10), op0=ALU.mult, op1=ALU.add)
                nc.sync.dma_start(C(12), inp["gamma"][si*128: si*128+p, :])
                nc.sync.dma_start(C(13), inp["beta"][si*128: si*128+p, :])
                nc.vector.tensor_mul(C(14), C(12), C(11))
                nc.vector.tensor_mul(C(15), C(2), C(14))
                nc.vector.tensor_sub(C(16), C(13), C(15))
            stp = [pers.tile([p_, 32], F32, tag=f"stp{si_}", name=f"stp{si_}")
                   for si_, p_ in ((0, 128), (1, 32))]
            for si in range(2):
                nc.vector.tensor_copy(stp[si][:, 0:17], stats[si][:, 0:17])

        # ---------------- phase 0b: W0 fold + gi0 window 0 ---------
        # ph0k (folded W0 + staged x) stays open for the whole program so
        # the pool stack stays LIFO; the fp32 fold temps live in ph0t,
        # which closes before the scan pools open.
        ph0k = ctx.enter_context(tc.tile_pool(name="ph0k", bufs=1))
        w0ab = ph0k.tile([128, 3072], BF16, tag="w0ab", name="w0ab")
        w0bb = ph0k.tile([32, 3072], BF16, tag="w0bb", name="w0bb")
        xto_a = ph0k.tile([128, ROWS], BF16, tag="xtoa", name="xtoa")
        nc.sync.dma_start(xto_a[:], inp["xto"][0:128, :])
        xto_b = ph0k.tile([32, ROWS], BF16, tag="xtob", name="xtob")
        nc.sync.dma_start(xto_b[:], inp["xto"][128:160, :])
        KB = CIN + 1 - 128  # 23 rows incl folded bias row

        with tc.tile_pool(name="ph0t", bufs=1) as ph0t:
            w0a = ph0t.tile([128, 3072], F32, tag="w0a", name="w0a")
            nc.sync.dma_start(w0a[:], inp["wih0a"][:])
            w0b = ph0t.tile([32, 3072], F32, tag="w0b", name="w0b")
            nc.sync.dma_start(w0b[:], inp["wih0b"][:])
            bias0 = ph0t.tile([1, 3072], F32, tag="bias0t", name="bias0t")
            nc.sync.dma_start(bias0[:], inp["bias0"][:])

            w0rows = [(w0a, 128, stp[0]), (w0b, CIN - 128, stp[1])]
            for n in range(6):
                bps = gi_psum_pool.tile([128, 512], F32, tag="gips",
                                        name=f"bps{n}")
                for ki, (w0, kp, st) in enumerate(w0rows):
                    nc.tensor.matmul(
                        bps[0:1, 0:512], st[0:kp, 16:17],
                        w0[0:kp, n * 512: (n + 1) * 512],
                        start=(ki == 0), stop=(ki == 1))
                nc.vector.tensor_add(bias0[:, n * 512: (n + 1) * 512],
                                     bps[0:1, 0:512],
                                     bias0[:, n * 512: (n + 1) * 512])
            for w0, kp, st in w0rows:
                nc.vector.tensor_scalar_mul(w0[0:kp, :], w0[0:kp, :],
                                            st[0:kp, 14:15])
            nc.sync.dma_start(w0b[CIN - 128: CIN - 128 + 1, :], bias0[:])
            nc.vector.tensor_copy(w0ab[:], w0a[:])
            nc.vector.tensor_copy(w0bb[0:KB, :], w0b[0:KB, :])

        scan_pool = ctx.enter_context(tc.tile_pool(name="scan", bufs=1))
        owin_pool = ctx.enter_context(tc.tile_pool(name="owin", bufs=2))
        og_pool = ctx.enter_context(tc.tile_pool(name="og", bufs=2))
        wih_pool = ctx.enter_context(tc.tile_pool(name="wih", bufs=2))

        copyout_q = []

        def gi0_job(g, kind, cc0, cw, r0, r1):
            mw = r1 - r0
            gps = gi_psum_pool.tile([128, 512], F32, tag="gips",
                                    name=f"g0ps{r0}_{g}_{kind}")
            nc.tensor.matmul(
                gps[0:mw, 0:cw], xto_a[:, r0:r1],
                w0ab[:, g * 768 + cc0: g * 768 + cc0 + cw],
                start=True, stop=False)
            nc.tensor.matmul(
                gps[0:mw, 0:cw], xto_b[0:KB, r0:r1],
                w0bb[0:KB, g * 768 + cc0: g * 768 + cc0 + cw],
                start=False, stop=True)

            def copy_out():
                gsb = gicp_pool.tile([128, cw], BF16, tag=f"gisb_{kind}",
                                     name=f"g0sb{r0}_{g}_{kind}")
                nc.scalar.copy(gsb[0:mw, :], gps[0:mw, 0:cw])
                store_gi(0, g, r0, r1, cc0, cw, gsb)
            copyout_q.append(copy_out)

        # upfront: gi0 for window 0 rows
        up_hi = min(windows[0][1] * B, ROWS)
        for (r0, r1) in mchunks(0, up_hi):
            for g in range(4):
                for (kind, cc0, cw) in (("rz", 0, 512), ("n", 512, 256)):
                    gi0_job(g, kind, cc0, cw, r0, r1)
                    while copyout_q:
                        copyout_q.pop(0)()

        # deferred gi0 jobs (drained in layer-0 scan gaps)
        pending = []  # (cost, min_gstep, fn, mid, seq)
        seq_ctr = [0]
        batch_seq = {}  # (gi_layer, window_idx) -> last seq of that batch

        def bump_seq(key):
            seq_ctr[0] += 1
            batch_seq[key] = seq_ctr[0]
            return seq_ctr[0]

        for wi in range(1, NW):
            ts, te = windows[wi]
            sq = bump_seq((0, wi))
            for (r0, r1) in mchunks(ts * B, te * B):
                for g in range(4):
                    for (kind, cc0, cw) in (("rz", 0, 512), ("n", 512, 256)):
                        pending.append((2 * MMC(cw) + 250, 0,
                                        (lambda g_=g, k_=kind, c_=cc0,
                                         w_=cw, a_=r0, b_=r1:
                                         gi0_job(g_, k_, c_, w_, a_, b_)),
                                        False, sq))

        # ---------------- scan setup ----------------
        outT_last = pers.tile([128, 256], BF16, tag="outTlast",
                              name="outT_last")

        xpack = scan_pool.tile([128, (WSTEPS // 2) * 128], BF16, tag="xpack",
                               name="xpack")

        def exchange_window(l, ts, te, owin):
            """AllGather packed own outT window with pair core.

            The b<16 pack runs on the DVE (strided SBUF copy) so the sync
            DMA queue only sees contiguous transfers, and the og unpack
            DMAs sit on the gpsimd queue behind the collective so their
            wait never blocks the next window's gi chunk loads."""
            wsz = (te - ts) * B
            tagsfx = f"{l}_{ts}"
            nsw = te - ts
            ow_v = owin.rearrange("p (s j b) -> p s j b", j=8, b=32)
            xin = xch_pool.tile([128, 8 * wsz], BF16, tag="xin",
                                name=f"xin_{tagsfx}")
            for j0 in (0, 4):
                nc.vector.tensor_copy(
                    xpack[:, 0: 4 * wsz].rearrange(
                        "p (j s b) -> p j s b", j=4, b=16),
                    ow_v[:, 0:nsw, j0:j0 + 4, 0:16].rearrange(
                        "p s j b -> p j s b"))
                nc.sync.dma_start(xin[:, j0 * wsz: (j0 + 4) * wsz],
                                  xpack[:, 0: 4 * wsz])
            xout = xch_pool.tile([2, 128, 8 * wsz], BF16, tag="xout",
                                 name=f"xout_{tagsfx}")
            nc.gpsimd.collective_compute(
                "AllGather", ALU.bypass, replica_groups=PAIRS,
                ins=[xin[:].opt()], outs=[xout[:].opt()])
            og = [og_pool.tile([128, 8 * wsz], BF16, tag=f"ogd{d}",
                               name=f"og{d}_{tagsfx}") for d in range(2)]
            for d in range(2):
                nc.gpsimd.dma_start(og[d][:, 0: 8 * wsz], xout[d])
            return og

        def exchange_last(owin2, slot):
            xin = xch_pool.tile([128, 128], BF16, tag="xinL", name="xin_last")
            src = owin2[:, slot * 256: (slot + 1) * 256].rearrange(
                "p (j b) -> p j b", j=8)[:, :, 0:16]
            nc.sync.dma_start(
                xin.rearrange("p (j b) -> p j b", j=8), src)
            xout = xch_pool.tile([2, 128, 128], BF16, tag="xoutL",
                                 name="xout_last")
            nc.gpsimd.collective_compute(
                "AllGather", ALU.bypass, replica_groups=PAIRS,
                ins=[xin[:].opt()], outs=[xout[:].opt()])
            for d in range(2):
                nc.gpsimd.dma_start(outT_last[:, d * 128: (d + 1) * 128],
                                    xout[d])

        def gi_jobs(l, ts, te, og, avail_gstep):
            """Sub-jobs for the layer-(l+1) gi GEMM of a window."""
            nsw = te - ts
            groups = [(g, kind, cc0, cw)
                      for g in range(4)
                      for (kind, cc0, cw) in (("rz", 0, 512),
                                              ("n", 512, 256))]
            tiles = {}
            psums = {}

            def load_job(gi_idx):
                g, kind, cc0, cw = groups[gi_idx]
                col = g * 768 + cc0
                wt = wih_pool.tile([128, 16 * cw], BF16, tag="wihT",
                                   name=f"wt_{l}{ts}{g}{kind}")
                for k in range(16):
                    nc.sync.dma_start(
                        wt[:, k * cw: (k + 1) * cw],
                        inp["wih12"][l, k, :, col: col + cw])
                bt = wih_pool.tile([1, cw], BF16, tag="biasT",
                                   name=f"bt_{l}{ts}{g}{kind}")
                nc.sync.dma_start(bt[:], inp["bias12"][l, :, col: col + cw])
                tiles[gi_idx] = (wt, bt)

            def mm_sub(gi_idx, r0, r1, k0, k1, last):
                g, kind, cc0, cw = groups[gi_idx]
                wt, bt = tiles[gi_idx]
                mw = r1 - r0
                q0 = r0 - ts * B
                wsz = (te - ts) * B
                if k0 == 0:
                    psums[(gi_idx, r0)] = gi_psum_pool.tile(
                        [128, 512], F32, tag="gips",
                        name=f"gp_{l}{ts}{g}{kind}{r0}")
                gps = psums[(gi_idx, r0)]
                for k in range(k0, k1):
                    dsrc, kk = k // 8, k % 8
                    lhsT = og[dsrc][:, kk * wsz + q0: kk * wsz + q0 + mw]
                    nc.tensor.matmul(
                        gps[0:mw, 0:cw], lhsT,
                        wt[:, k * cw: (k + 1) * cw],
                        start=(k == 0), stop=False)
                if last:
                    nc.tensor.matmul(
                        gps[0:mw, 0:cw], ones_bf[:, 0:mw], bt[:],
                        start=False, stop=True)

                    def copy_out():
                        if DEBUG_EMIT and l == 1:
                            print(f"EMIT store l2 g={g} kind={kind} "
                                  f"r0={r0} gstep={gstep[0]}")
                        gsb = gicp_pool.tile(
                            [128, cw], BF16, tag=f"gisb_{kind}",
                            name=f"gs_{l}{ts}{g}{kind}{r0}")
                        nc.scalar.copy(gsb[0:mw, :], gps[0:mw, 0:cw])
                        store_gi(l + 1, g, r0, r1, cc0, cw, gsb)
                        del psums[(gi_idx, r0)]
                    copyout_q.append(copy_out)

            jobs = []
            rcs = mchunks(ts * B, te * B)
            mg = avail_gstep + 4
            sq = bump_seq((l + 1, ts // WSTEPS))
            jobs.append((150, mg, lambda: load_job(0), False, sq))
            for gi_idx in range(len(groups)):
                g, kind, cc0, cw = groups[gi_idx]
                if gi_idx + 1 < len(groups):
                    jobs.append((150, mg, lambda i=gi_idx + 1: load_job(i),
                                 False, sq))
                ksp = ((0, 4), (4, 8), (8, 12), (12, 16)) if kind == "rz" \
                    else ((0, 8), (8, 16))
                for (r0, r1) in rcs:
                    for (k0, k1) in ksp:
                        cost = (k1 - k0) * MMC(cw) + (300 if k1 == 16 else 0)
                        jobs.append((cost, mg,
                                     lambda i=gi_idx, a=r0, b=r1, x=k0, y=k1,
                                     z=(k1 == 16): mm_sub(i, a, b, x, y, z),
                                     k0 > 0, sq))
            return jobs

        # ---------------- layers ----------------
        drained = [0.0]
        gstep = [0]

        def drain_through(key):
            """Emit all pending jobs up to and including batch `key` so the
            gi DRAM rows a chunk-load reads are stored first."""
            tgt = batch_seq.get(key)
            if tgt is None:
                return
            while pending and pending[0][4] <= tgt:
                e = pending.pop(0)
                e[2]()
                drained[0] += e[0]
            while copyout_q:
                copyout_q.pop(0)()

        h_elem = [scan_pool.tile([128, 256], BF16, tag=f"h_{par}",
                                 name=f"h_{par}") for par in range(2)]
        zlhs = scan_pool.tile([128, B], BF16, tag="zlhs", name="zlhs")
        nc.vector.memset(zlhs[:], 0.0)
        scr = scan_pool.tile([128, 2048], BF16, tag="scr", name="scr")
        owin2 = scan_pool.tile([128, 512], BF16, tag="owin2", name="owin2")

        for l in range(3):
            whh_sb = scan_pool.tile([128, 8 * 3072], BF16, tag="whh_sb",
                                    name=f"whh_sb{l}")
            for q in range(4):
                nc.scalar.dma_start(whh_sb[:, q * 6144: (q + 1) * 6144],
                                    inp["whh"][l][:, q * 6144: (q + 1) * 6144])
            bhhn_sb = scan_pool.tile([1, 1024], BF16, tag="bhhn_sb",
                                     name=f"bhhn_sb{l}")
            nc.scalar.dma_start(bhhn_sb[:], inp["bhhn"][l])

            nc.vector.memset(h_elem[0][:], 0.0)

            windows_l = windows if l < 2 else [(0, T)]

            chunk_tiles = {}

            def load_chunk(t0, te):
                if DEBUG_EMIT and l == 2:
                    print(f"EMIT load l2 t0={t0} gstep={gstep[0]}")
                tS = min(SCH, te - t0)
                drain_through((l, (t0 + tS - 1) // WSTEPS))
                grz = gld_pool.tile([128, SCH * 512], BF16, tag="grz",
                                    name=f"grz_{l}_{t0}")
                gst = gld_pool.tile([128, SCH * 256], BF16, tag="gst",
                                    name=f"gst_{l}_{t0}")
                if SIM_SAFE:
                    nc.vector.memset(grz[:], 0.0)
                    nc.vector.memset(gst[:], 0.0)
                for g in range(4):
                    nc.sync.dma_start(
                        grz[32*g: 32*g + B, 0: tS * 512],
                        gi_rz[l][t0: t0 + tS, g].rearrange(
                            "s b c -> b s c"))
                    nc.sync.dma_start(
                        gst[32*g: 32*g + B, 0: tS * 256],
                        gi_n[l][t0: t0 + tS, g].rearrange(
                            "s b c -> b s c"))
                chunk_tiles[t0] = (grz, gst)
                return grz, gst

            def emit_pre(t, ts):
                """ident-MM gi_rz preload + n-bias MM for step t (PSUM)."""
                so = (t - ts) % SCH
                grz, _ = chunk_tiles[ts + ((t - ts) // SCH) * SCH]
                gp = gates_pool.tile([128, 768], F32, tag="gp",
                                     name=f"gp_{l}_{t}")
                for g in range(4):
                    nc.tensor.matmul(
                        gp[32*g: 32*g + 32, 0:512],
                        identrep[32*g: 32*g + 16, 0:32],
                        grz[32*g: 32*g + 16, so * 512: (so + 1) * 512],
                        start=True, stop=False,
                        skip_group_check=True,
                        tile_position=(32 * g, 32 * g))
                for g in range(4):
                    nc.tensor.matmul(
                        gp[32*g: 32*g + 32, 512:768],
                        ones_bf[:, 0:32],
                        bhhn_sb[:, g * 256: (g+1) * 256],
                        start=True, stop=False,
                        skip_group_check=True,
                        tile_position=(0, 32 * g))
                return gp

            owin_prev, nsteps_prev = None, 0
            for (ts, te) in windows_l:
                nsteps = te - ts
                if l < 2:
                    owin = owin_pool.tile([128, nsteps * 256], BF16,
                                          tag="owin", name=f"owin_{l}_{ts}")
                else:
                    owin = owin2
                # chunk prefetch: first two chunks of this window
                load_chunk(ts, te)
                if ts + SCH < te:
                    load_chunk(ts + SCH, te)
                gp_next = emit_pre(ts, ts)

                for t in range(ts, te):
                    so = (t - ts) % SCH
                    gp = gp_next
                    grz_c, gst_c = chunk_tiles[ts + ((t - ts) // SCH) * SCH]

                    def lhsT_for(j):
                        if t == 0:
                            return zlhs[:, 0:B]
                        if l < 2:
                            if t == ts:
                                return owin_prev[
                                    :, (nsteps_prev - 1) * 256 + j * 32:
                                    (nsteps_prev - 1) * 256 + j * 32 + 16]
                            return owin[:, (t - 1 - ts) * 256 + j * 32:
                                        (t - 1 - ts) * 256 + j * 32 + 16]
                        return owin2[:, ((t - 1) % 2) * 256 + j * 32:
                                     ((t - 1) % 2) * 256 + j * 32 + 16]

                    for k in range(8):
                        lhsT = lhsT_for(k)
                        for (c0, cw) in ((0, 512), (512, 256)):
                            for g in range(4):
                                nc.tensor.matmul(
                                    gp[32*g: 32*g + B, c0: c0 + cw],
                                    lhsT,
                                    whh_sb[:, k * 3072 + g * 768 + c0:
                                           k * 3072 + g * 768 + c0 + cw],
                                    start=False,
                                    stop=(k == 7),
                                    skip_group_check=True,
                                    tile_position=(0, 32 * g))

                    # issue next chunk prefetch at each chunk boundary
                    # (gld bufs=2: the reused buf's reads were all emitted
                    # during the chunk before last)
                    if so == 0 and t > ts and t + SCH < te:
                        load_chunk(t + SCH, te)
                    # preload for next step (runs in this step's PE gap)
                    if t + 1 < te:
                        gp_next = emit_pre(t + 1, ts)

                    # drain gi jobs into the PE gap per the global plan
                    nemit = 0
                    allowed = cumtgt[gstep[0]]
                    while (pending and drained[0] < allowed
                           and pending[0][1] <= gstep[0]):
                        cost, _, fn, _mid, _sq = pending.pop(0)
                        fn()
                        drained[0] += cost
                        nemit += 1
                    # finish an in-flight job's remaining sub-jobs before
                    # anything else can recycle its PSUM accumulator
                    while (pending and pending[0][3]
                           and pending[0][1] <= gstep[0]):
                        cost, _, fn, _mid, _sq = pending.pop(0)
                        fn()
                        drained[0] += cost
                        nemit += 1
                    if nemit == 0 and t - ts >= 2:
                        wps = gi_psum_pool.tile([128, 512], F32, tag="gips",
                                                name=f"warm_{l}_{t}")
                        for wq in range(3):
                            nc.tensor.matmul(
                                wps[0:B, 0:512], zlhs[:, 0:B],
                                whh_sb[:, wq * 512: (wq + 1) * 512],
                                start=(wq == 0), stop=(wq == 2))

                    # ---- activation chain, split into cj halves ----
                    h_prev = h_elem[t % 2]
                    h_new = h_elem[(t + 1) % 2]
                    rz = scr[:, 0:512]
                    omz = scr[:, 512:768]
                    t1 = scr[:, 768:1024]
                    t2 = scr[:, 1024:1280]
                    nt = scr[:, 1280:1536]
                    zh = scr[:, 1536:1792]
                    u = scr[:, 1792:2048]

                    gp_rz = gp[:, 0:512].rearrange(
                        "p (two c) -> p two c", two=2)
                    rz_v = rz.rearrange("p (two c) -> p two c", two=2)

                    def half(h0):
                        h1 = h0 + 128
                        nc.scalar.activation(
                            rz_v[:, :, h0:h1], gp_rz[:, :, h0:h1],
                            AF.Sigmoid)
                        nc.vector.tensor_mul(
                            t1[:, h0:h1], rz[:, h0:h1],
                            gp[:, 512 + h0: 512 + h1])
                        nc.vector.tensor_add(
                            t2[:, h0:h1], t1[:, h0:h1],
                            gst_c[:, so * 256 + h0: so * 256 + h1])
                        nc.scalar.activation(nt[:, h0:h1], t2[:, h0:h1],
                                             AF.Tanh)
                        nc.vector.tensor_scalar(
                            omz[:, h0:h1], rz[:, 256 + h0: 256 + h1],
                            -1.0, 1.0, op0=ALU.mult, op1=ALU.add)
                        nc.vector.tensor_mul(
                            zh[:, h0:h1], rz[:, 256 + h0: 256 + h1],
                            h_prev[:, h0:h1])

                    def tail(h0):
                        h1 = h0 + 128
                        nc.vector.tensor_mul(u[:, h0:h1], omz[:, h0:h1],
                                             nt[:, h0:h1])
                        nc.vector.tensor_add(h_new[:, h0:h1], u[:, h0:h1],
                                             zh[:, h0:h1])

                    half(0)
                    half(128)
                    tail(0)
                    tail(128)

                    if l < 2:
                        dst = owin[:, (t - ts) * 256: (t - ts + 1) * 256]
                    else:
                        dst = owin2[:, (t % 2) * 256: (t % 2 + 1) * 256]
                    nc.vector.transpose(dst, h_new[:])

                    while copyout_q:
                        copyout_q.pop(0)()
                    gstep[0] += 1

                # end of window
                if l < 2:
                    og = exchange_window(l, ts, te, owin)
                    pending.extend(gi_jobs(l, ts, te, og, gstep[0]))
                owin_prev, nsteps_prev = owin, nsteps

        # flush any gi jobs not drained during step gaps
        while pending:
            pending.pop(0)[2]()  # noqa
        while copyout_q:
            copyout_q.pop(0)()

        exchange_last(owin2, (T - 1) % 2)

        # ---------------- FC ----------------
        fcw = pers.tile([128, 16 * NCLS], BF16, tag="fcw")
        for k in range(16):
            nc.sync.dma_start(fcw[:, k * NCLS: (k + 1) * NCLS],
                              inp["fcwT"][k])
        fcb = pers.tile([1, NCLS], BF16, tag="fcb")
        nc.sync.dma_start(fcb[:], inp["fcb"][:])
        fps = gi_psum_pool.tile([128, 512], F32, tag="gips", name="fps")
        for k in range(16):
            nc.tensor.matmul(
                fps[0:B, 0:NCLS],
                outT_last[:, k * 16: k * 16 + 16],
                fcw[:, k * NCLS: (k + 1) * NCLS],
                start=(k == 0), stop=False)
        nc.tensor.matmul(fps[0:B, 0:NCLS], ones_bf[:, 0:B], fcb[:],
                         start=False, stop=True)
        fout = gicp_pool.tile([B, NCLS], F32, tag="fout")
        nc.vector.tensor_copy(fout[:], fps[0:B, 0:NCLS])
        nc.sync.dma_start(out_t[:], fout[:])

        ctx.close()

    nc.compile()
    return nc


_cache = {}


def kernel(**inputs):
    T = inputs["x"].shape[1]
    n_full = inputs["x"].shape[0]
    key = ("prog", T, n_full)
    if key not in _cache:
        _cache[key] = build_program(T, n_full)
    nc = _cache[key]
    per_core = host_prep(inputs, T, n_full)
    res = run_bass_kernel_spmd(nc, per_core, core_ids=list(range(NCORES)))
    out = np.concatenate([res.results[2 * s]["out"] for s in range(NSHARD)],
                         axis=0)
    return np.ascontiguousarray(out.astype(np.float32))
